# revision 20
# baseline (speedup 1.0000x reference)
"""AttnBlock (GroupNorm + single-head self-attention + residual) on 8 TRN2
NeuronCores.

Reference computation (per image b of 4, tokens N=64*64=4096, C=512):
    hn  = GroupNorm(x)  (32 groups, eps 1e-6, affine)
    q,k,v = hn @ wq + bq, ...
    attn = softmax(q @ k.T / sqrt(C)); out = attn @ v
    y   = x + out @ wo + bo

Sharding: one NeuronCore per (image, half): core 2b+h computes attention
rows [h*2048, (h+1)*2048) of image b. Each core redundantly computes
GroupNorm stats and full-image K/V (cheap vs. cross-core collectives) and
its own 2048 query rows. No inter-core communication.

Per-core layout: everything feature-major ([C, tokens]) so every matmul
contraction sits on the partition axis; the final projection naturally
returns to row-major. The host pre-transposes/casts x to bf16
feature-major per core (shard prep) and passes the residual rows in f32.

Device pipeline:
  1. GroupNorm statistics via bn_stats/bn_aggr on the raw bf16 x
     (feature-major; per-channel over tokens, then group-combined with a
     block-diagonal averaging matmul).
  2. The normalization hn = x*A + B is FOLDED INTO THE QKV WEIGHTS:
     W' = A∘W (row scale), b' = b + B@W. The projections then consume the
     raw x tiles directly - no normalize pass on the critical path.
  3. Attention: scoresT = kT^T q (feature-major both sides), exp on the
     Scalar engine without max subtraction (scores provably in [-2,2] for
     unit-normalized inputs), softmax denominator accumulated on the
     Vector engine, attn@v and output projection on TensorE, with the
     1/denominator applied per query row in the epilogue (softmax
     normalization commutes with the linear attn@v and output proj).
Compute dtype: bf16 operands, f32 PSUM accumulation.
"""

import sys

if "/opt/trn_rl_repo" not in sys.path:
    sys.path.insert(0, "/opt/trn_rl_repo")

import numpy as np
import ml_dtypes

import concourse.bass as bass
import concourse.tile as tile
from concourse import bacc, mybir
from concourse.bass_utils import run_bass_kernel_spmd

F32 = mybir.dt.float32
BF16 = mybir.dt.bfloat16

B, H, W, C = 4, 64, 64, 512
N_TOK = H * W            # tokens per image
NQ = N_TOK // 2          # query rows per core
G = 32                   # groups
GS = C // G              # channels per group (16)
EPS = 1e-6
SCALE = float(C) ** -0.5
CT = C // 128            # channel tiles (4)
JT = N_TOK // 128        # token tiles (32)
IB = NQ // 512           # query i-blocks (4)

_CACHE = {}


def _build():
    nc = bacc.Bacc("TRN2", target_bir_lowering=False)

    xt_e = nc.dram_tensor("xt", [C, N_TOK], BF16, kind="ExternalInput")
    xr_e = nc.dram_tensor("xr", [NQ, C], F32, kind="ExternalInput")
    w_e = {
        n: nc.dram_tensor(n, [C, C], BF16, kind="ExternalInput")
        for n in ("wq", "wk", "wv", "wo")
    }
    b_e = {
        n: nc.dram_tensor(n, [C], F32, kind="ExternalInput")
        for n in ("bq", "bk", "bv")
    }
    gs_e = nc.dram_tensor("gsc", [C], F32, kind="ExternalInput")
    gb_e = nc.dram_tensor("gbi", [C], F32, kind="ExternalInput")
    gm_e = nc.dram_tensor("gmat", [128, 128], F32, kind="ExternalInput")
    out_e = nc.dram_tensor("out", [NQ, C], F32, kind="ExternalOutput")

    def col(e):  # [C] dram -> [C,1] view for partition-major loads
        return e.ap().rearrange("(a b) -> a b", b=1)

    with tile.TileContext(nc) as tc:
        with (
            tc.tile_pool(name="const", bufs=1) as const,
            tc.tile_pool(name="big", bufs=1) as big,
            tc.tile_pool(name="stat", bufs=1) as stat,
            tc.tile_pool(name="ework", bufs=4) as ework,
            tc.tile_pool(name="attw", bufs=8) as attw,
            tc.tile_pool(name="owork", bufs=4) as owork,
            tc.tile_pool(name="xrw", bufs=3) as xrw,
            tc.tile_pool(name="rdenw", bufs=8) as rdenw,
        ):
            # ---- x feature-major (bf16) ----
            # whole-tile contiguous DMAs (1 MB each; strided sub-chunk loads
            # are an order of magnitude slower), issued before the weight
            # loads so the stats-critical data arrives first
            xT = []
            for k in range(CT):
                t = big.tile([128, N_TOK], BF16, tag=f"xT{k}", name=f"xT{k}")
                for hh in range(2):
                    nc.sync.dma_start(
                        out=t[:, hh * 2048:(hh + 1) * 2048],
                        in_=xt_e.ap()[k * 128:(k + 1) * 128,
                                      hh * 2048:(hh + 1) * 2048],
                    )
                xT.append(t)

            # ---- weights / constants ----
            wsb = {}
            for n in ("wq", "wk", "wv", "wo"):
                wsb[n] = []
                for k in range(CT):
                    t = const.tile([128, C], BF16, tag=f"w_{n}_{k}")
                    nc.sync.dma_start(out=t, in_=w_e[n].ap()[k * 128:(k + 1) * 128, :])
                    wsb[n].append(t)
            bsb = {}
            for n in ("bq", "bk"):
                bsb[n] = []
                for m in range(CT):
                    t = const.tile([128, 1], F32, tag=f"b_{n}_{m}")
                    nc.sync.dma_start(out=t, in_=col(b_e[n])[m * 128:(m + 1) * 128, :])
                    bsb[n].append(t)
            bvb = const.tile([128, C], F32, tag="bvb")
            nc.sync.dma_start(
                out=bvb,
                in_=bass.AP(tensor=b_e["bv"], offset=0, ap=[[0, 128], [1, C]]),
            )
            gssb, gbsb = [], []
            for m in range(CT):
                t = const.tile([128, 1], F32, tag=f"gs_{m}")
                nc.sync.dma_start(out=t, in_=col(gs_e)[m * 128:(m + 1) * 128, :])
                gssb.append(t)
                t = const.tile([128, 1], F32, tag=f"gb_{m}")
                nc.sync.dma_start(out=t, in_=col(gb_e)[m * 128:(m + 1) * 128, :])
                gbsb.append(t)
            gm_sb = const.tile([128, 128], F32, tag="gmat")
            nc.sync.dma_start(out=gm_sb, in_=gm_e.ap())
            ones_bcol = const.tile([1, 128], F32, tag="ones_bcol")
            nc.vector.memset(ones_bcol, 1.0)
            onef = const.tile([1, 1], F32, tag="onef")
            nc.vector.memset(onef, 1.0)
            onesf_col = const.tile([128, 1], F32, tag="onesf_col")
            nc.vector.memset(onesf_col, 1.0)
            epst = const.tile([128, 1], F32, tag="epst")
            nc.vector.memset(epst, EPS)

            # ---- GroupNorm stats + weight folding ----
            with tc.tile_pool(name="ps_misc", bufs=2, space="PSUM") as psm:
                Af, Bbf = [], []   # A (f32 [128,1]); B cast to bf16 for matmuls
                for k in range(CT):
                    stats = stat.tile([128, 8, 6], F32, tag=f"st{k}")
                    for ch in range(8):
                        nc.vector.bn_stats(
                            out=stats[:, ch, :],
                            in_=xT[k][:, ch * 512:(ch + 1) * 512],
                        )
                    mv = stat.tile([128, 2], F32, tag=f"mv{k}")
                    nc.vector.bn_aggr(out=mv, in_=stats)
                    # sm = (mean, var + mean^2) per channel
                    sm = stat.tile([128, 2], F32, tag=f"sm{k}")
                    nc.vector.tensor_copy(out=sm[:, 0:1], in_=mv[:, 0:1])
                    nc.vector.tensor_mul(out=sm[:, 1:2], in0=mv[:, 0:1], in1=mv[:, 0:1])
                    nc.vector.tensor_add(out=sm[:, 1:2], in0=sm[:, 1:2], in1=mv[:, 1:2])
                    # group-average via block-diagonal (1/GS) matrix
                    gps = psm.tile([128, 2], F32, tag="gps")
                    nc.tensor.matmul(gps, gm_sb, sm, start=True, stop=True)
                    gsb = stat.tile([128, 2], F32, tag=f"gsb{k}")
                    nc.vector.tensor_copy(out=gsb, in_=gps)
                    # var_g = E_g[x^2]-mean_g^2; A = rstd*scale; B = bias-mean_g*A
                    msq = stat.tile([128, 1], F32, tag=f"msq{k}")
                    nc.vector.tensor_mul(out=msq, in0=gsb[:, 0:1], in1=gsb[:, 0:1])
                    varg = stat.tile([128, 1], F32, tag=f"vg{k}")
                    nc.vector.tensor_sub(out=varg, in0=gsb[:, 1:2], in1=msq)
                    sd = stat.tile([128, 1], F32, tag=f"sd{k}")
                    nc.scalar.activation(
                        out=sd, in_=varg,
                        func=mybir.ActivationFunctionType.Sqrt,
                        bias=epst, scale=1.0,
                    )
                    rstd = stat.tile([128, 1], F32, tag=f"rs{k}")
                    nc.vector.reciprocal(out=rstd, in_=sd)
                    At = stat.tile([128, 1], F32, tag=f"A{k}")
                    nc.vector.tensor_mul(out=At, in0=rstd, in1=gssb[k])
                    mA = stat.tile([128, 1], F32, tag=f"mA{k}")
                    nc.vector.tensor_mul(out=mA, in0=gsb[:, 0:1], in1=At)
                    Bt = stat.tile([128, 1], F32, tag=f"B{k}")
                    nc.vector.tensor_sub(out=Bt, in0=gbsb[k], in1=mA)
                    Bb = stat.tile([128, 1], BF16, tag=f"Bb{k}")
                    nc.vector.tensor_copy(out=Bb, in_=Bt)
                    Af.append(At)
                    Bbf.append(Bb)

                # fold normalization into weights:
                #   W' = A (row) ∘ W ;  b' = b + B @ W
                # B@W matmuls read the ORIGINAL W (Tile orders them before the
                # in-place row scale below via WAR deps).
                badj = {}
                for n in ("wq", "wk", "wv"):
                    pb = psm.tile([1, 512], F32, tag="pb", name=f"pb_{n}")
                    for k in range(CT):
                        nc.tensor.matmul(
                            pb, Bbf[k], wsb[n][k],
                            start=(k == 0), stop=(k == CT - 1),
                        )
                    bs_ = stat.tile([1, 512], F32, tag=f"badj_{n}")
                    nc.vector.tensor_copy(out=bs_, in_=pb)
                    badj[n] = bs_
                # row-scale the weights on the Scalar engine (idle here) into
                # SEPARATE tiles: an in-place fold would WAR-depend on the
                # b-adjust matmuls above (which need the last tile's stats),
                # serializing the whole prologue. Separate outputs let the
                # k-th fold fire as soon as A[k] is ready, so QKV matmuls
                # start ~25us earlier.
                wf = {}
                for n in ("wq", "wk", "wv"):
                    wf[n] = []
                    for k in range(CT):
                        t = const.tile([128, C], BF16, tag=f"wf_{n}_{k}",
                                       name=f"wf_{n}_{k}")
                        nc.scalar.activation(
                            out=t, in_=wsb[n][k],
                            func=mybir.ActivationFunctionType.Copy,
                            scale=Af[k],
                        )
                        wf[n].append(t)
                wf["wo"] = wsb["wo"]
                # transpose b' pieces to per-partition layout for q/k;
                # build broadcast bias for v.
                bqf, bkf = [], []
                for n, dst in (("wq", bqf), ("wk", bkf)):
                    for m in range(CT):
                        pt = psm.tile([128, 1], F32, tag="pt", name=f"pt_{n}{m}")
                        nc.tensor.matmul(
                            pt, badj[n][0:1, m * 128:(m + 1) * 128], onef,
                            start=True, stop=True,
                        )
                        bf = stat.tile([128, 1], F32, tag=f"bf_{n}{m}")
                        base = bsb["bq"][m] if n == "wq" else bsb["bk"][m]
                        nc.vector.tensor_add(out=bf, in0=pt, in1=base)
                        dst.append(bf)
                pvb = psm.tile([128, 512], F32, tag="pvb")
                nc.tensor.matmul(pvb, ones_bcol, badj["wv"], start=True, stop=True)
                nc.vector.tensor_add(out=bvb, in0=pvb, in1=bvb)

            # ---- projections (raw x in, folded weights) ----
            kT = [big.tile([128, N_TOK], BF16, tag=f"kT{m}", name=f"kT{m}")
                  for m in range(CT)]
            qT = [big.tile([128, NQ], BF16, tag=f"qT{m}", name=f"qT{m}")
                  for m in range(CT)]
            v_sb = big.tile([128, JT, C], BF16, tag="v")
            with tc.tile_pool(name="ps_proj", bufs=6, space="PSUM") as psp:
                for m in range(CT):
                    for nt in range(N_TOK // 512):
                        pk = psp.tile([128, 512], F32, tag="p")
                        for k in range(CT):
                            nc.tensor.matmul(
                                pk,
                                wf["wk"][k][:, m * 128:(m + 1) * 128],
                                xT[k][:, nt * 512:(nt + 1) * 512],
                                start=(k == 0), stop=(k == CT - 1),
                            )
                        nc.vector.tensor_scalar_add(
                            out=kT[m][:, nt * 512:(nt + 1) * 512],
                            in0=pk, scalar1=bkf[m],
                        )
                    for nt in range(NQ // 512):
                        pq = psp.tile([128, 512], F32, tag="p")
                        for k in range(CT):
                            nc.tensor.matmul(
                                pq,
                                wf["wq"][k][:, m * 128:(m + 1) * 128],
                                xT[k][:, nt * 512:(nt + 1) * 512],
                                start=(k == 0), stop=(k == CT - 1),
                            )
                        nc.vector.tensor_scalar_add(
                            out=qT[m][:, nt * 512:(nt + 1) * 512],
                            in0=pq, scalar1=bqf[m],
                        )
                for jt in range(JT):
                    pv = psp.tile([128, 512], F32, tag="p")
                    for k in range(CT):
                        nc.tensor.matmul(
                            pv,
                            xT[k][:, jt * 128:(jt + 1) * 128],
                            wf["wv"][k],
                            start=(k == 0), stop=(k == CT - 1),
                        )
                    nc.vector.tensor_add(out=v_sb[:, jt, :], in0=pv, in1=bvb)

            # ---- attention ----
            with (
                tc.tile_pool(name="ps_att", bufs=4, space="PSUM") as psa,
                tc.tile_pool(name="ps_s", bufs=2, space="PSUM") as pss,
                tc.tile_pool(name="ps_o", bufs=1, space="PSUM") as pso,
                tc.tile_pool(name="ps_den", bufs=1, space="PSUM") as psd,
            ):
                for ib in range(IB):
                    att_ps = [psa.tile([128, 512], F32, tag="att", name=f"att_ps{cs}")
                              for cs in range(CT)]
                    dacc = owork.tile([128, 512], F32, tag="dacc")
                    nc.vector.memset(dacc, 0.0)
                    for jt in range(JT):
                        s_ps = pss.tile([128, 512], F32, tag="s")
                        for k in range(CT):
                            nc.tensor.matmul(
                                s_ps,
                                kT[k][:, jt * 128:(jt + 1) * 128],
                                qT[k][:, ib * 512:(ib + 1) * 512],
                                start=(k == 0), stop=(k == CT - 1),
                            )
                        e_t = ework.tile([128, 512], BF16, tag="e")
                        nc.scalar.activation(
                            out=e_t, in_=s_ps,
                            func=mybir.ActivationFunctionType.Exp,
                            scale=SCALE,
                        )
                        for cs in range(CT):
                            nc.tensor.matmul(
                                att_ps[cs],
                                v_sb[:, jt, cs * 128:(cs + 1) * 128],
                                e_t,
                                start=(jt == 0), stop=(jt == JT - 1),
                            )
                        nc.vector.tensor_add(out=dacc, in0=dacc, in1=e_t)
                    # denominator: column sums of dacc (over j partitions)
                    den_ps = psd.tile([1, 512], F32, tag="den")
                    nc.tensor.matmul(den_ps, onesf_col, dacc, start=True, stop=True)
                    attT = []
                    for cs in range(CT):
                        t = attw.tile([128, 512], BF16, tag="attT", name=f"attT{cs}")
                        nc.vector.tensor_copy(out=t, in_=att_ps[cs])
                        attT.append(t)
                    den_sb = owork.tile([1, 512], F32, tag="den_sb")
                    nc.vector.tensor_copy(out=den_sb, in_=den_ps)
                    for it in range(4):
                        row0 = (ib * 4 + it) * 128
                        dT = pso.tile([128, 1], F32, tag="o",
                                      padded_shape=[128, 512], name=f"dT{it}")
                        nc.tensor.matmul(
                            dT, den_sb[0:1, it * 128:(it + 1) * 128], onef,
                            start=True, stop=True,
                        )
                        rden = rdenw.tile([128, 1], F32, tag="rden")
                        nc.vector.reciprocal(out=rden, in_=dT)
                        o_ps = pso.tile([128, 512], F32, tag="o", name=f"o_ps{it}")
                        for cs in range(CT):
                            nc.tensor.matmul(
                                o_ps,
                                attT[cs][:, it * 128:(it + 1) * 128],
                                wsb["wo"][cs],
                                start=(cs == 0), stop=(cs == CT - 1),
                            )
                        xr_t = xrw.tile([128, C], F32, tag="xr")
                        nc.sync.dma_start(
                            out=xr_t, in_=xr_e.ap()[row0:row0 + 128, :]
                        )
                        o_t = owork.tile([128, C], F32, tag="o")
                        nc.vector.scalar_tensor_tensor(
                            out=o_t, in0=o_ps, scalar=rden, in1=xr_t,
                            op0=mybir.AluOpType.mult, op1=mybir.AluOpType.add,
                        )
                        nc.sync.dma_start(
                            out=out_e.ap()[row0:row0 + 128, :], in_=o_t
                        )

    nc.compile()
    return nc


def _get_nc():
    if "nc" not in _CACHE:
        _CACHE["nc"] = _build()
    return _CACHE["nc"]


def kernel(**inputs) -> np.ndarray:
    x = np.asarray(inputs["x"], dtype=np.float32)          # [B,H,W,C]
    gn_scale = np.asarray(inputs["gn_scale"], np.float32)
    gn_bias = np.asarray(inputs["gn_bias"], np.float32)
    ws = {n: np.ascontiguousarray(
        np.asarray(inputs[n], np.float32).astype(ml_dtypes.bfloat16))
        for n in ("wq", "wk", "wv", "wo")}
    bs = {n: np.asarray(inputs[n], np.float32) for n in ("bq", "bk", "bv", "bo")}

    gmat = np.zeros((128, 128), np.float32)
    for g in range(128 // GS):
        gmat[g * GS:(g + 1) * GS, g * GS:(g + 1) * GS] = 1.0 / GS

    xf = x.reshape(B, N_TOK, C)
    in_maps = []
    for core in range(8):
        b, h = divmod(core, 2)
        own = xf[b, h * NQ:(h + 1) * NQ]          # [NQ, C] fp32
        other = xf[b, (1 - h) * NQ:(2 - h) * NQ]
        perm = np.concatenate([own, other], axis=0)        # own half first
        xt = np.ascontiguousarray(perm.T.astype(ml_dtypes.bfloat16))  # [C, N]
        xr = np.ascontiguousarray(own + bs["bo"][None, :])  # residual (+bo)
        in_maps.append({
            "xt": xt,
            "xr": xr,
            "wq": ws["wq"], "wk": ws["wk"], "wv": ws["wv"], "wo": ws["wo"],
            "bq": bs["bq"], "bk": bs["bk"], "bv": bs["bv"],
            "gsc": gn_scale, "gbi": gn_bias,
            "gmat": gmat,
        })

    nc = _get_nc()
    res = run_bass_kernel_spmd(nc, in_maps, core_ids=list(range(8)))

    out = np.empty((B, N_TOK, C), np.float32)
    for core in range(8):
        b, h = divmod(core, 2)
        out[b, h * NQ:(h + 1) * NQ] = res.results[core]["out"]
    return out.reshape(B, H, W, C)


# revision 21
# speedup vs baseline: 1.1842x; 1.1842x over previous
"""AttnBlock (GroupNorm + single-head self-attention + residual) on 8 TRN2
NeuronCores.

Reference computation (per image b of 4, tokens N=64*64=4096, C=512):
    hn  = GroupNorm(x)  (32 groups, eps 1e-6, affine)
    q,k,v = hn @ wq + bq, ...
    attn = softmax(q @ k.T / sqrt(C)); out = attn @ v
    y   = x + out @ wo + bo

Sharding: one NeuronCore per (image, half): core 2b+h computes attention
rows [h*2048, (h+1)*2048) of image b. Each core redundantly computes
GroupNorm stats and full-image K/V (cheap vs. cross-core collectives) and
its own 2048 query rows. No inter-core communication.

Per-core layout: everything feature-major ([C, tokens]) so every matmul
contraction sits on the partition axis; the final projection naturally
returns to row-major. The host pre-transposes/casts x to bf16
feature-major per core (shard prep) and passes the residual rows in f32.

Device pipeline:
  1. GroupNorm statistics via bn_stats/bn_aggr on the raw bf16 x
     (feature-major; per-channel over tokens, then group-combined with a
     block-diagonal averaging matmul).
  2. The normalization hn = x*A + B is FOLDED INTO THE QKV WEIGHTS:
     W' = A∘W (row scale), b' = b + B@W. The projections then consume the
     raw x tiles directly - no normalize pass on the critical path.
  3. Attention: scoresT = kT^T q (feature-major both sides), exp on the
     Scalar engine without max subtraction (scores provably in [-2,2] for
     unit-normalized inputs), softmax denominator accumulated on the
     Vector engine, attn@v and output projection on TensorE, with the
     1/denominator applied per query row in the epilogue (softmax
     normalization commutes with the linear attn@v and output proj).
Compute dtype: bf16 operands, f32 PSUM accumulation.
"""

import sys

if "/opt/trn_rl_repo" not in sys.path:
    sys.path.insert(0, "/opt/trn_rl_repo")

import numpy as np
import ml_dtypes

import concourse.bass as bass
import concourse.tile as tile
from concourse import bacc, mybir
from concourse.bass_utils import run_bass_kernel_spmd

F32 = mybir.dt.float32
BF16 = mybir.dt.bfloat16

B, H, W, C = 4, 64, 64, 512
N_TOK = H * W            # tokens per image
NQ = N_TOK // 2          # query rows per core
G = 32                   # groups
GS = C // G              # channels per group (16)
EPS = 1e-6
SCALE = float(C) ** -0.5
CT = C // 128            # channel tiles (4)
JT = N_TOK // 128        # token tiles (32)
IB = NQ // 512           # query i-blocks (4)

_CACHE = {}


def _build():
    nc = bacc.Bacc("TRN2", target_bir_lowering=False)

    xt_e = nc.dram_tensor("xt", [C, N_TOK], BF16, kind="ExternalInput")
    xr_e = nc.dram_tensor("xr", [NQ, C], F32, kind="ExternalInput")
    w_e = {
        n: nc.dram_tensor(n, [C, C], BF16, kind="ExternalInput")
        for n in ("wq", "wk", "wv", "wo")
    }
    b_e = {
        n: nc.dram_tensor(n, [C], F32, kind="ExternalInput")
        for n in ("bq", "bk", "bv")
    }
    gs_e = nc.dram_tensor("gsc", [C], F32, kind="ExternalInput")
    gb_e = nc.dram_tensor("gbi", [C], F32, kind="ExternalInput")
    gm_e = nc.dram_tensor("gmat", [128, 128], F32, kind="ExternalInput")
    out_e = nc.dram_tensor("out", [NQ, C], F32, kind="ExternalOutput")

    def col(e):  # [C] dram -> [C,1] view for partition-major loads
        return e.ap().rearrange("(a b) -> a b", b=1)

    with tile.TileContext(nc) as tc:
        with (
            tc.tile_pool(name="const", bufs=1) as const,
            tc.tile_pool(name="big", bufs=1) as big,
            tc.tile_pool(name="stat", bufs=1) as stat,
            tc.tile_pool(name="ework", bufs=4) as ework,
            tc.tile_pool(name="attw", bufs=8) as attw,
            tc.tile_pool(name="owork", bufs=4) as owork,
            tc.tile_pool(name="xrw", bufs=3) as xrw,
            tc.tile_pool(name="rdenw", bufs=8) as rdenw,
        ):
            # ---- x feature-major (bf16) ----
            # whole-tile contiguous DMAs (1 MB each; strided sub-chunk loads
            # are an order of magnitude slower), issued before the weight
            # loads so the stats-critical data arrives first
            xT = []
            for k in range(CT):
                t = big.tile([128, N_TOK], BF16, tag=f"xT{k}", name=f"xT{k}")
                for hh in range(2):
                    nc.sync.dma_start(
                        out=t[:, hh * 2048:(hh + 1) * 2048],
                        in_=xt_e.ap()[k * 128:(k + 1) * 128,
                                      hh * 2048:(hh + 1) * 2048],
                    )
                xT.append(t)

            # ---- weights / constants ----
            wsb = {}
            for n in ("wq", "wk", "wv", "wo"):
                wsb[n] = []
                for k in range(CT):
                    t = const.tile([128, C], BF16, tag=f"w_{n}_{k}")
                    nc.sync.dma_start(out=t, in_=w_e[n].ap()[k * 128:(k + 1) * 128, :])
                    wsb[n].append(t)
            bsb = {}
            for n in ("bq", "bk"):
                bsb[n] = []
                for m in range(CT):
                    t = const.tile([128, 1], F32, tag=f"b_{n}_{m}")
                    nc.sync.dma_start(out=t, in_=col(b_e[n])[m * 128:(m + 1) * 128, :])
                    bsb[n].append(t)
            bvb = const.tile([128, C], F32, tag="bvb")
            nc.sync.dma_start(
                out=bvb,
                in_=bass.AP(tensor=b_e["bv"], offset=0, ap=[[0, 128], [1, C]]),
            )
            gssb, gbsb = [], []
            for m in range(CT):
                t = const.tile([128, 1], F32, tag=f"gs_{m}")
                nc.sync.dma_start(out=t, in_=col(gs_e)[m * 128:(m + 1) * 128, :])
                gssb.append(t)
                t = const.tile([128, 1], F32, tag=f"gb_{m}")
                nc.sync.dma_start(out=t, in_=col(gb_e)[m * 128:(m + 1) * 128, :])
                gbsb.append(t)
            gm_sb = const.tile([128, 128], F32, tag="gmat")
            nc.sync.dma_start(out=gm_sb, in_=gm_e.ap())
            ones_bcol = const.tile([1, 128], F32, tag="ones_bcol")
            nc.vector.memset(ones_bcol, 1.0)
            onef = const.tile([1, 1], F32, tag="onef")
            nc.vector.memset(onef, 1.0)
            onesf_col = const.tile([128, 1], F32, tag="onesf_col")
            nc.vector.memset(onesf_col, 1.0)
            epst = const.tile([128, 1], F32, tag="epst")
            nc.vector.memset(epst, EPS)

            # ---- GroupNorm stats + weight folding + projections ----
            # ps_misc (4 tags x 1 buf = 4 banks) and ps_proj (4 banks) are
            # OPEN SIMULTANEOUSLY: a stacked open/close would make the
            # projection pool's banks WAR-depend on the whole stats phase.
            with (
                tc.tile_pool(name="ps_misc", bufs=1, space="PSUM") as psm,
                tc.tile_pool(name="ps_proj", bufs=4, space="PSUM") as psp,
            ):
                Af, Bbf = [], []   # A (f32 [128,1]); B cast to bf16 for matmuls
                for k in range(CT):
                    stats = stat.tile([128, 8, 6], F32, tag=f"st{k}")
                    for ch in range(8):
                        nc.vector.bn_stats(
                            out=stats[:, ch, :],
                            in_=xT[k][:, ch * 512:(ch + 1) * 512],
                        )
                    mv = stat.tile([128, 2], F32, tag=f"mv{k}")
                    nc.vector.bn_aggr(out=mv, in_=stats)
                    # sm = (mean, var + mean^2) per channel
                    sm = stat.tile([128, 2], F32, tag=f"sm{k}")
                    nc.vector.tensor_copy(out=sm[:, 0:1], in_=mv[:, 0:1])
                    nc.vector.tensor_mul(out=sm[:, 1:2], in0=mv[:, 0:1], in1=mv[:, 0:1])
                    nc.vector.tensor_add(out=sm[:, 1:2], in0=sm[:, 1:2], in1=mv[:, 1:2])
                    # group-average via block-diagonal (1/GS) matrix
                    gps = psm.tile([128, 2], F32, tag="gps")
                    nc.tensor.matmul(gps, gm_sb, sm, start=True, stop=True)
                    gsb = stat.tile([128, 2], F32, tag=f"gsb{k}")
                    nc.vector.tensor_copy(out=gsb, in_=gps)
                    # var_g = E_g[x^2]-mean_g^2; A = rstd*scale; B = bias-mean_g*A
                    msq = stat.tile([128, 1], F32, tag=f"msq{k}")
                    nc.vector.tensor_mul(out=msq, in0=gsb[:, 0:1], in1=gsb[:, 0:1])
                    varg = stat.tile([128, 1], F32, tag=f"vg{k}")
                    nc.vector.tensor_sub(out=varg, in0=gsb[:, 1:2], in1=msq)
                    sd = stat.tile([128, 1], F32, tag=f"sd{k}")
                    nc.scalar.activation(
                        out=sd, in_=varg,
                        func=mybir.ActivationFunctionType.Sqrt,
                        bias=epst, scale=1.0,
                    )
                    rstd = stat.tile([128, 1], F32, tag=f"rs{k}")
                    nc.vector.reciprocal(out=rstd, in_=sd)
                    At = stat.tile([128, 1], F32, tag=f"A{k}")
                    nc.vector.tensor_mul(out=At, in0=rstd, in1=gssb[k])
                    mA = stat.tile([128, 1], F32, tag=f"mA{k}")
                    nc.vector.tensor_mul(out=mA, in0=gsb[:, 0:1], in1=At)
                    Bt = stat.tile([128, 1], F32, tag=f"B{k}")
                    nc.vector.tensor_sub(out=Bt, in0=gbsb[k], in1=mA)
                    Bb = stat.tile([128, 1], BF16, tag=f"Bb{k}")
                    nc.vector.tensor_copy(out=Bb, in_=Bt)
                    Af.append(At)
                    Bbf.append(Bb)

                # fold normalization into weights:
                #   W' = A (row) ∘ W ;  b' = b + B @ W
                # B@W matmuls read the ORIGINAL W (Tile orders them before the
                # in-place row scale below via WAR deps).
                badj = {}
                for n in ("wq", "wk", "wv"):
                    pb = psm.tile([1, 512], F32, tag="pb", name=f"pb_{n}")
                    for k in range(CT):
                        nc.tensor.matmul(
                            pb, Bbf[k], wsb[n][k],
                            start=(k == 0), stop=(k == CT - 1),
                        )
                    bs_ = stat.tile([1, 512], F32, tag=f"badj_{n}")
                    nc.vector.tensor_copy(out=bs_, in_=pb)
                    badj[n] = bs_
                # row-scale the weights on the Scalar engine (idle here) into
                # SEPARATE tiles: an in-place fold would WAR-depend on the
                # b-adjust matmuls above (which need the last tile's stats),
                # serializing the whole prologue. Separate outputs let the
                # k-th fold fire as soon as A[k] is ready, so QKV matmuls
                # start ~25us earlier.
                wf = {}
                for n in ("wq", "wk", "wv"):
                    wf[n] = []
                    for k in range(CT):
                        t = const.tile([128, C], BF16, tag=f"wf_{n}_{k}",
                                       name=f"wf_{n}_{k}")
                        nc.scalar.activation(
                            out=t, in_=wsb[n][k],
                            func=mybir.ActivationFunctionType.Copy,
                            scale=Af[k],
                        )
                        wf[n].append(t)
                wf["wo"] = wsb["wo"]
                # transpose b' pieces to per-partition layout for q/k;
                # build broadcast bias for v.
                bqf, bkf = [], []
                for n, dst in (("wq", bqf), ("wk", bkf)):
                    for m in range(CT):
                        pt = psm.tile([128, 1], F32, tag="pt", name=f"pt_{n}{m}")
                        nc.tensor.matmul(
                            pt, badj[n][0:1, m * 128:(m + 1) * 128], onef,
                            start=True, stop=True,
                        )
                        bf = stat.tile([128, 1], F32, tag=f"bf_{n}{m}")
                        base = bsb["bq"][m] if n == "wq" else bsb["bk"][m]
                        nc.vector.tensor_add(out=bf, in0=pt, in1=base)
                        dst.append(bf)
                pvb = psm.tile([128, 512], F32, tag="pvb")
                nc.tensor.matmul(pvb, ones_bcol, badj["wv"], start=True, stop=True)
                nc.vector.tensor_add(out=bvb, in0=pvb, in1=bvb)

                # ---- projections (raw x in, folded weights) ----
                kT = [big.tile([128, N_TOK], BF16, tag=f"kT{m}", name=f"kT{m}")
                      for m in range(CT)]
                qT = [big.tile([128, NQ], BF16, tag=f"qT{m}", name=f"qT{m}")
                      for m in range(CT)]
                v_sb = big.tile([128, JT, C], BF16, tag="v")
                for m in range(CT):
                    for nt in range(N_TOK // 512):
                        pk = psp.tile([128, 512], F32, tag="p")
                        for k in range(CT):
                            nc.tensor.matmul(
                                pk,
                                wf["wk"][k][:, m * 128:(m + 1) * 128],
                                xT[k][:, nt * 512:(nt + 1) * 512],
                                start=(k == 0), stop=(k == CT - 1),
                            )
                        nc.vector.tensor_scalar_add(
                            out=kT[m][:, nt * 512:(nt + 1) * 512],
                            in0=pk, scalar1=bkf[m],
                        )
                    for nt in range(NQ // 512):
                        pq = psp.tile([128, 512], F32, tag="p")
                        for k in range(CT):
                            nc.tensor.matmul(
                                pq,
                                wf["wq"][k][:, m * 128:(m + 1) * 128],
                                xT[k][:, nt * 512:(nt + 1) * 512],
                                start=(k == 0), stop=(k == CT - 1),
                            )
                        nc.vector.tensor_scalar_add(
                            out=qT[m][:, nt * 512:(nt + 1) * 512],
                            in0=pq, scalar1=bqf[m],
                        )
                for jt in range(JT):
                    pv = psp.tile([128, 512], F32, tag="p")
                    for k in range(CT):
                        nc.tensor.matmul(
                            pv,
                            xT[k][:, jt * 128:(jt + 1) * 128],
                            wf["wv"][k],
                            start=(k == 0), stop=(k == CT - 1),
                        )
                    nc.vector.tensor_add(out=v_sb[:, jt, :], in0=pv, in1=bvb)

            # ---- attention ----
            with (
                tc.tile_pool(name="ps_att", bufs=4, space="PSUM") as psa,
                tc.tile_pool(name="ps_s", bufs=2, space="PSUM") as pss,
                tc.tile_pool(name="ps_o", bufs=1, space="PSUM") as pso,
                tc.tile_pool(name="ps_den", bufs=1, space="PSUM") as psd,
            ):
                for ib in range(IB):
                    att_ps = [psa.tile([128, 512], F32, tag="att", name=f"att_ps{cs}")
                              for cs in range(CT)]
                    dacc = owork.tile([128, 512], F32, tag="dacc")
                    nc.vector.memset(dacc, 0.0)
                    for jt in range(JT):
                        s_ps = pss.tile([128, 512], F32, tag="s")
                        for k in range(CT):
                            nc.tensor.matmul(
                                s_ps,
                                kT[k][:, jt * 128:(jt + 1) * 128],
                                qT[k][:, ib * 512:(ib + 1) * 512],
                                start=(k == 0), stop=(k == CT - 1),
                            )
                        e_t = ework.tile([128, 512], BF16, tag="e")
                        nc.scalar.activation(
                            out=e_t, in_=s_ps,
                            func=mybir.ActivationFunctionType.Exp,
                            scale=SCALE,
                        )
                        for cs in range(CT):
                            nc.tensor.matmul(
                                att_ps[cs],
                                v_sb[:, jt, cs * 128:(cs + 1) * 128],
                                e_t,
                                start=(jt == 0), stop=(jt == JT - 1),
                            )
                        nc.vector.tensor_add(out=dacc, in0=dacc, in1=e_t)
                    # denominator: column sums of dacc (over j partitions)
                    den_ps = psd.tile([1, 512], F32, tag="den")
                    nc.tensor.matmul(den_ps, onesf_col, dacc, start=True, stop=True)
                    attT = []
                    for cs in range(CT):
                        t = attw.tile([128, 512], BF16, tag="attT", name=f"attT{cs}")
                        nc.vector.tensor_copy(out=t, in_=att_ps[cs])
                        attT.append(t)
                    den_sb = owork.tile([1, 512], F32, tag="den_sb")
                    nc.vector.tensor_copy(out=den_sb, in_=den_ps)
                    for it in range(4):
                        row0 = (ib * 4 + it) * 128
                        dT = pso.tile([128, 1], F32, tag="o",
                                      padded_shape=[128, 512], name=f"dT{it}")
                        nc.tensor.matmul(
                            dT, den_sb[0:1, it * 128:(it + 1) * 128], onef,
                            start=True, stop=True,
                        )
                        rden = rdenw.tile([128, 1], F32, tag="rden")
                        nc.vector.reciprocal(out=rden, in_=dT)
                        o_ps = pso.tile([128, 512], F32, tag="o", name=f"o_ps{it}")
                        for cs in range(CT):
                            nc.tensor.matmul(
                                o_ps,
                                attT[cs][:, it * 128:(it + 1) * 128],
                                wsb["wo"][cs],
                                start=(cs == 0), stop=(cs == CT - 1),
                            )
                        xr_t = xrw.tile([128, C], F32, tag="xr")
                        nc.sync.dma_start(
                            out=xr_t, in_=xr_e.ap()[row0:row0 + 128, :]
                        )
                        o_t = owork.tile([128, C], F32, tag="o")
                        nc.vector.scalar_tensor_tensor(
                            out=o_t, in0=o_ps, scalar=rden, in1=xr_t,
                            op0=mybir.AluOpType.mult, op1=mybir.AluOpType.add,
                        )
                        nc.sync.dma_start(
                            out=out_e.ap()[row0:row0 + 128, :], in_=o_t
                        )

    nc.compile()
    return nc


def _get_nc():
    if "nc" not in _CACHE:
        _CACHE["nc"] = _build()
    return _CACHE["nc"]


def kernel(**inputs) -> np.ndarray:
    x = np.asarray(inputs["x"], dtype=np.float32)          # [B,H,W,C]
    gn_scale = np.asarray(inputs["gn_scale"], np.float32)
    gn_bias = np.asarray(inputs["gn_bias"], np.float32)
    ws = {n: np.ascontiguousarray(
        np.asarray(inputs[n], np.float32).astype(ml_dtypes.bfloat16))
        for n in ("wq", "wk", "wv", "wo")}
    bs = {n: np.asarray(inputs[n], np.float32) for n in ("bq", "bk", "bv", "bo")}

    gmat = np.zeros((128, 128), np.float32)
    for g in range(128 // GS):
        gmat[g * GS:(g + 1) * GS, g * GS:(g + 1) * GS] = 1.0 / GS

    xf = x.reshape(B, N_TOK, C)
    in_maps = []
    for core in range(8):
        b, h = divmod(core, 2)
        own = xf[b, h * NQ:(h + 1) * NQ]          # [NQ, C] fp32
        other = xf[b, (1 - h) * NQ:(2 - h) * NQ]
        perm = np.concatenate([own, other], axis=0)        # own half first
        xt = np.ascontiguousarray(perm.T.astype(ml_dtypes.bfloat16))  # [C, N]
        xr = np.ascontiguousarray(own + bs["bo"][None, :])  # residual (+bo)
        in_maps.append({
            "xt": xt,
            "xr": xr,
            "wq": ws["wq"], "wk": ws["wk"], "wv": ws["wv"], "wo": ws["wo"],
            "bq": bs["bq"], "bk": bs["bk"], "bv": bs["bv"],
            "gsc": gn_scale, "gbi": gn_bias,
            "gmat": gmat,
        })

    nc = _get_nc()
    res = run_bass_kernel_spmd(nc, in_maps, core_ids=list(range(8)))

    out = np.empty((B, N_TOK, C), np.float32)
    for core in range(8):
        b, h = divmod(core, 2)
        out[b, h * NQ:(h + 1) * NQ] = res.results[core]["out"]
    return out.reshape(B, H, W, C)


# revision 22
# speedup vs baseline: 1.2130x; 1.0243x over previous
"""AttnBlock (GroupNorm + single-head self-attention + residual) on 8 TRN2
NeuronCores.

Reference computation (per image b of 4, tokens N=64*64=4096, C=512):
    hn  = GroupNorm(x)  (32 groups, eps 1e-6, affine)
    q,k,v = hn @ wq + bq, ...
    attn = softmax(q @ k.T / sqrt(C)); out = attn @ v
    y   = x + out @ wo + bo

Sharding: one NeuronCore per (image, half): core 2b+h computes attention
rows [h*2048, (h+1)*2048) of image b. Each core redundantly computes
GroupNorm stats and full-image K/V (cheap vs. cross-core collectives) and
its own 2048 query rows. No inter-core communication.

Per-core layout: everything feature-major ([C, tokens]) so every matmul
contraction sits on the partition axis; the final projection naturally
returns to row-major. The host pre-transposes/casts x to bf16
feature-major per core (shard prep) and passes the residual rows in f32.

Device pipeline:
  1. GroupNorm statistics via bn_stats/bn_aggr on the raw bf16 x
     (feature-major; per-channel over tokens, then group-combined with a
     block-diagonal averaging matmul).
  2. The normalization hn = x*A + B is FOLDED INTO THE QKV WEIGHTS:
     W' = A∘W (row scale), b' = b + B@W. The projections then consume the
     raw x tiles directly - no normalize pass on the critical path.
  3. Attention: scoresT = kT^T q (feature-major both sides), exp on the
     Scalar engine without max subtraction (scores provably in [-2,2] for
     unit-normalized inputs), softmax denominator accumulated on the
     Vector engine, attn@v and output projection on TensorE, with the
     1/denominator applied per query row in the epilogue (softmax
     normalization commutes with the linear attn@v and output proj).
Compute dtype: bf16 operands, f32 PSUM accumulation.
"""

import sys

if "/opt/trn_rl_repo" not in sys.path:
    sys.path.insert(0, "/opt/trn_rl_repo")

import numpy as np
import ml_dtypes

import concourse.bass as bass
import concourse.tile as tile
from concourse import bacc, mybir
from concourse.bass_utils import run_bass_kernel_spmd

F32 = mybir.dt.float32
BF16 = mybir.dt.bfloat16

B, H, W, C = 4, 64, 64, 512
N_TOK = H * W            # tokens per image
NQ = N_TOK // 2          # query rows per core
G = 32                   # groups
GS = C // G              # channels per group (16)
EPS = 1e-6
SCALE = float(C) ** -0.5
CT = C // 128            # channel tiles (4)
JT = N_TOK // 128        # token tiles (32)
IB = NQ // 512           # query i-blocks (4)

_CACHE = {}


def _build():
    nc = bacc.Bacc("TRN2", target_bir_lowering=False)

    xt_e = nc.dram_tensor("xt", [C, N_TOK], BF16, kind="ExternalInput")
    xr_e = nc.dram_tensor("xr", [NQ, C], F32, kind="ExternalInput")
    w_e = {
        n: nc.dram_tensor(n, [C, C], BF16, kind="ExternalInput")
        for n in ("wq", "wk", "wv", "wo")
    }
    b_e = {
        n: nc.dram_tensor(n, [C], F32, kind="ExternalInput")
        for n in ("bq", "bk", "bv")
    }
    gs_e = nc.dram_tensor("gsc", [C], F32, kind="ExternalInput")
    gb_e = nc.dram_tensor("gbi", [C], F32, kind="ExternalInput")
    gm_e = nc.dram_tensor("gmat", [128, 128], F32, kind="ExternalInput")
    out_e = nc.dram_tensor("out", [NQ, C], F32, kind="ExternalOutput")

    def col(e):  # [C] dram -> [C,1] view for partition-major loads
        return e.ap().rearrange("(a b) -> a b", b=1)

    with tile.TileContext(nc) as tc:
        with (
            tc.tile_pool(name="const", bufs=1) as const,
            tc.tile_pool(name="big", bufs=1) as big,
            tc.tile_pool(name="stat", bufs=1) as stat,
            tc.tile_pool(name="ework", bufs=4) as ework,
            tc.tile_pool(name="attw", bufs=8) as attw,
            tc.tile_pool(name="owork", bufs=4) as owork,
            tc.tile_pool(name="xrw", bufs=3) as xrw,
            tc.tile_pool(name="rdenw", bufs=8) as rdenw,
        ):
            # ---- DMA issue order = need order: the per-queue completion
            # counters are cumulative, so anything queued behind a large
            # transfer waits for it. gmat/biases gate the first PE op;
            # x gates stats; weights are needed only ~20us in.
            gm_sb = const.tile([128, 128], F32, tag="gmat")
            nc.sync.dma_start(out=gm_sb, in_=gm_e.ap())

            # ---- x feature-major (bf16) ----
            # half-tile contiguous DMAs (512 KB each)
            xT = []
            for k in range(CT):
                t = big.tile([128, N_TOK], BF16, tag=f"xT{k}", name=f"xT{k}")
                for hh in range(2):
                    nc.sync.dma_start(
                        out=t[:, hh * 2048:(hh + 1) * 2048],
                        in_=xt_e.ap()[k * 128:(k + 1) * 128,
                                      hh * 2048:(hh + 1) * 2048],
                    )
                xT.append(t)

            bsb = {}
            for n in ("bq", "bk"):
                bsb[n] = []
                for m in range(CT):
                    t = const.tile([128, 1], F32, tag=f"b_{n}_{m}")
                    nc.sync.dma_start(out=t, in_=col(b_e[n])[m * 128:(m + 1) * 128, :])
                    bsb[n].append(t)
            bvb = const.tile([128, C], F32, tag="bvb")
            nc.sync.dma_start(
                out=bvb,
                in_=bass.AP(tensor=b_e["bv"], offset=0, ap=[[0, 128], [1, C]]),
            )
            gssb, gbsb = [], []
            for m in range(CT):
                t = const.tile([128, 1], F32, tag=f"gs_{m}")
                nc.sync.dma_start(out=t, in_=col(gs_e)[m * 128:(m + 1) * 128, :])
                gssb.append(t)
                t = const.tile([128, 1], F32, tag=f"gb_{m}")
                nc.sync.dma_start(out=t, in_=col(gb_e)[m * 128:(m + 1) * 128, :])
                gbsb.append(t)
            # ---- weights (largest, least urgent) ----
            wsb = {}
            for n in ("wq", "wk", "wv", "wo"):
                wsb[n] = []
                for k in range(CT):
                    t = const.tile([128, C], BF16, tag=f"w_{n}_{k}", name=f"w_{n}_{k}")
                    nc.sync.dma_start(out=t, in_=w_e[n].ap()[k * 128:(k + 1) * 128, :])
                    wsb[n].append(t)
            ones_bcol = const.tile([1, 128], F32, tag="ones_bcol")
            nc.vector.memset(ones_bcol, 1.0)
            onef = const.tile([1, 1], F32, tag="onef")
            nc.vector.memset(onef, 1.0)
            onesf_col = const.tile([128, 1], F32, tag="onesf_col")
            nc.vector.memset(onesf_col, 1.0)
            epst = const.tile([128, 1], F32, tag="epst")
            nc.vector.memset(epst, EPS)

            # ---- GroupNorm stats + weight folding + projections ----
            # ps_misc (4 tags x 1 buf = 4 banks) and ps_proj (4 banks) are
            # OPEN SIMULTANEOUSLY: a stacked open/close would make the
            # projection pool's banks WAR-depend on the whole stats phase.
            with (
                tc.tile_pool(name="ps_misc", bufs=1, space="PSUM") as psm,
                tc.tile_pool(name="ps_proj", bufs=4, space="PSUM") as psp,
            ):
                Af, Bbf = [], []   # A (f32 [128,1]); B cast to bf16 for matmuls
                for k in range(CT):
                    stats = stat.tile([128, 8, 6], F32, tag=f"st{k}")
                    for ch in range(8):
                        nc.vector.bn_stats(
                            out=stats[:, ch, :],
                            in_=xT[k][:, ch * 512:(ch + 1) * 512],
                        )
                    mv = stat.tile([128, 2], F32, tag=f"mv{k}")
                    nc.vector.bn_aggr(out=mv, in_=stats)
                    # sm = (mean, var + mean^2) per channel
                    sm = stat.tile([128, 2], F32, tag=f"sm{k}")
                    nc.vector.tensor_copy(out=sm[:, 0:1], in_=mv[:, 0:1])
                    nc.vector.tensor_mul(out=sm[:, 1:2], in0=mv[:, 0:1], in1=mv[:, 0:1])
                    nc.vector.tensor_add(out=sm[:, 1:2], in0=sm[:, 1:2], in1=mv[:, 1:2])
                    # group-average via block-diagonal (1/GS) matrix
                    gps = psm.tile([128, 2], F32, tag="gps")
                    nc.tensor.matmul(gps, gm_sb, sm, start=True, stop=True)
                    gsb = stat.tile([128, 2], F32, tag=f"gsb{k}")
                    nc.vector.tensor_copy(out=gsb, in_=gps)
                    # var_g = E_g[x^2]-mean_g^2; A = rstd*scale; B = bias-mean_g*A
                    msq = stat.tile([128, 1], F32, tag=f"msq{k}")
                    nc.vector.tensor_mul(out=msq, in0=gsb[:, 0:1], in1=gsb[:, 0:1])
                    varg = stat.tile([128, 1], F32, tag=f"vg{k}")
                    nc.vector.tensor_sub(out=varg, in0=gsb[:, 1:2], in1=msq)
                    sd = stat.tile([128, 1], F32, tag=f"sd{k}")
                    nc.scalar.activation(
                        out=sd, in_=varg,
                        func=mybir.ActivationFunctionType.Sqrt,
                        bias=epst, scale=1.0,
                    )
                    rstd = stat.tile([128, 1], F32, tag=f"rs{k}")
                    nc.vector.reciprocal(out=rstd, in_=sd)
                    At = stat.tile([128, 1], F32, tag=f"A{k}")
                    nc.vector.tensor_mul(out=At, in0=rstd, in1=gssb[k])
                    mA = stat.tile([128, 1], F32, tag=f"mA{k}")
                    nc.vector.tensor_mul(out=mA, in0=gsb[:, 0:1], in1=At)
                    Bt = stat.tile([128, 1], F32, tag=f"B{k}")
                    nc.vector.tensor_sub(out=Bt, in0=gbsb[k], in1=mA)
                    Bb = stat.tile([128, 1], BF16, tag=f"Bb{k}")
                    nc.vector.tensor_copy(out=Bb, in_=Bt)
                    Af.append(At)
                    Bbf.append(Bb)

                # fold normalization into weights:
                #   W' = A (row) ∘ W ;  b' = b + B @ W
                # B@W matmuls read the ORIGINAL W (Tile orders them before the
                # in-place row scale below via WAR deps).
                badj = {}
                for n in ("wq", "wk", "wv"):
                    pb = psm.tile([1, 512], F32, tag="pb", name=f"pb_{n}")
                    for k in range(CT):
                        nc.tensor.matmul(
                            pb, Bbf[k], wsb[n][k],
                            start=(k == 0), stop=(k == CT - 1),
                        )
                    bs_ = stat.tile([1, 512], F32, tag=f"badj_{n}")
                    nc.vector.tensor_copy(out=bs_, in_=pb)
                    badj[n] = bs_
                # row-scale the weights on the Scalar engine (idle here) into
                # SEPARATE tiles: an in-place fold would WAR-depend on the
                # b-adjust matmuls above (which need the last tile's stats),
                # serializing the whole prologue. Separate outputs let the
                # k-th fold fire as soon as A[k] is ready, so QKV matmuls
                # start ~25us earlier.
                wf = {}
                for n in ("wq", "wk", "wv"):
                    wf[n] = []
                    for k in range(CT):
                        t = const.tile([128, C], BF16, tag=f"wf_{n}_{k}",
                                       name=f"wf_{n}_{k}")
                        nc.scalar.activation(
                            out=t, in_=wsb[n][k],
                            func=mybir.ActivationFunctionType.Copy,
                            scale=Af[k],
                        )
                        wf[n].append(t)
                wf["wo"] = wsb["wo"]
                # transpose b' pieces to per-partition layout for q/k;
                # build broadcast bias for v.
                bqf, bkf = [], []
                for n, dst in (("wq", bqf), ("wk", bkf)):
                    for m in range(CT):
                        pt = psm.tile([128, 1], F32, tag="pt", name=f"pt_{n}{m}")
                        nc.tensor.matmul(
                            pt, badj[n][0:1, m * 128:(m + 1) * 128], onef,
                            start=True, stop=True,
                        )
                        bf = stat.tile([128, 1], F32, tag=f"bf_{n}{m}")
                        base = bsb["bq"][m] if n == "wq" else bsb["bk"][m]
                        nc.vector.tensor_add(out=bf, in0=pt, in1=base)
                        dst.append(bf)
                pvb = psm.tile([128, 512], F32, tag="pvb")
                nc.tensor.matmul(pvb, ones_bcol, badj["wv"], start=True, stop=True)
                nc.vector.tensor_add(out=bvb, in0=pvb, in1=bvb)

                # ---- projections (raw x in, folded weights) ----
                kT = [big.tile([128, N_TOK], BF16, tag=f"kT{m}", name=f"kT{m}")
                      for m in range(CT)]
                qT = [big.tile([128, NQ], BF16, tag=f"qT{m}", name=f"qT{m}")
                      for m in range(CT)]
                v_sb = big.tile([128, JT, C], BF16, tag="v")
                for m in range(CT):
                    for nt in range(N_TOK // 512):
                        pk = psp.tile([128, 512], F32, tag="p")
                        for k in range(CT):
                            nc.tensor.matmul(
                                pk,
                                wf["wk"][k][:, m * 128:(m + 1) * 128],
                                xT[k][:, nt * 512:(nt + 1) * 512],
                                start=(k == 0), stop=(k == CT - 1),
                            )
                        nc.vector.tensor_scalar_add(
                            out=kT[m][:, nt * 512:(nt + 1) * 512],
                            in0=pk, scalar1=bkf[m],
                        )
                    for nt in range(NQ // 512):
                        pq = psp.tile([128, 512], F32, tag="p")
                        for k in range(CT):
                            nc.tensor.matmul(
                                pq,
                                wf["wq"][k][:, m * 128:(m + 1) * 128],
                                xT[k][:, nt * 512:(nt + 1) * 512],
                                start=(k == 0), stop=(k == CT - 1),
                            )
                        nc.vector.tensor_scalar_add(
                            out=qT[m][:, nt * 512:(nt + 1) * 512],
                            in0=pq, scalar1=bqf[m],
                        )
                for jt in range(JT):
                    pv = psp.tile([128, 512], F32, tag="p")
                    for k in range(CT):
                        nc.tensor.matmul(
                            pv,
                            xT[k][:, jt * 128:(jt + 1) * 128],
                            wf["wv"][k],
                            start=(k == 0), stop=(k == CT - 1),
                        )
                    nc.vector.tensor_add(out=v_sb[:, jt, :], in0=pv, in1=bvb)

            # ---- attention ----
            with (
                tc.tile_pool(name="ps_att", bufs=4, space="PSUM") as psa,
                tc.tile_pool(name="ps_s", bufs=2, space="PSUM") as pss,
                tc.tile_pool(name="ps_o", bufs=1, space="PSUM") as pso,
                tc.tile_pool(name="ps_den", bufs=1, space="PSUM") as psd,
            ):
                for ib in range(IB):
                    att_ps = [psa.tile([128, 512], F32, tag="att", name=f"att_ps{cs}")
                              for cs in range(CT)]
                    dacc = owork.tile([128, 512], F32, tag="dacc")
                    nc.vector.memset(dacc, 0.0)
                    for jt in range(JT):
                        s_ps = pss.tile([128, 512], F32, tag="s")
                        for k in range(CT):
                            nc.tensor.matmul(
                                s_ps,
                                kT[k][:, jt * 128:(jt + 1) * 128],
                                qT[k][:, ib * 512:(ib + 1) * 512],
                                start=(k == 0), stop=(k == CT - 1),
                            )
                        e_t = ework.tile([128, 512], BF16, tag="e")
                        nc.scalar.activation(
                            out=e_t, in_=s_ps,
                            func=mybir.ActivationFunctionType.Exp,
                            scale=SCALE,
                        )
                        for cs in range(CT):
                            nc.tensor.matmul(
                                att_ps[cs],
                                v_sb[:, jt, cs * 128:(cs + 1) * 128],
                                e_t,
                                start=(jt == 0), stop=(jt == JT - 1),
                            )
                        nc.vector.tensor_add(out=dacc, in0=dacc, in1=e_t)
                    # denominator: column sums of dacc (over j partitions)
                    den_ps = psd.tile([1, 512], F32, tag="den")
                    nc.tensor.matmul(den_ps, onesf_col, dacc, start=True, stop=True)
                    attT = []
                    for cs in range(CT):
                        t = attw.tile([128, 512], BF16, tag="attT", name=f"attT{cs}")
                        nc.vector.tensor_copy(out=t, in_=att_ps[cs])
                        attT.append(t)
                    den_sb = owork.tile([1, 512], F32, tag="den_sb")
                    nc.vector.tensor_copy(out=den_sb, in_=den_ps)
                    for it in range(4):
                        row0 = (ib * 4 + it) * 128
                        dT = pso.tile([128, 1], F32, tag="o",
                                      padded_shape=[128, 512], name=f"dT{it}")
                        nc.tensor.matmul(
                            dT, den_sb[0:1, it * 128:(it + 1) * 128], onef,
                            start=True, stop=True,
                        )
                        rden = rdenw.tile([128, 1], F32, tag="rden")
                        nc.vector.reciprocal(out=rden, in_=dT)
                        o_ps = pso.tile([128, 512], F32, tag="o", name=f"o_ps{it}")
                        for cs in range(CT):
                            nc.tensor.matmul(
                                o_ps,
                                attT[cs][:, it * 128:(it + 1) * 128],
                                wsb["wo"][cs],
                                start=(cs == 0), stop=(cs == CT - 1),
                            )
                        xr_t = xrw.tile([128, C], F32, tag="xr")
                        nc.sync.dma_start(
                            out=xr_t, in_=xr_e.ap()[row0:row0 + 128, :]
                        )
                        o_t = owork.tile([128, C], F32, tag="o")
                        nc.vector.scalar_tensor_tensor(
                            out=o_t, in0=o_ps, scalar=rden, in1=xr_t,
                            op0=mybir.AluOpType.mult, op1=mybir.AluOpType.add,
                        )
                        nc.sync.dma_start(
                            out=out_e.ap()[row0:row0 + 128, :], in_=o_t
                        )

    nc.compile()
    return nc


def _get_nc():
    if "nc" not in _CACHE:
        _CACHE["nc"] = _build()
    return _CACHE["nc"]


def kernel(**inputs) -> np.ndarray:
    x = np.asarray(inputs["x"], dtype=np.float32)          # [B,H,W,C]
    gn_scale = np.asarray(inputs["gn_scale"], np.float32)
    gn_bias = np.asarray(inputs["gn_bias"], np.float32)
    ws = {n: np.ascontiguousarray(
        np.asarray(inputs[n], np.float32).astype(ml_dtypes.bfloat16))
        for n in ("wq", "wk", "wv", "wo")}
    bs = {n: np.asarray(inputs[n], np.float32) for n in ("bq", "bk", "bv", "bo")}

    gmat = np.zeros((128, 128), np.float32)
    for g in range(128 // GS):
        gmat[g * GS:(g + 1) * GS, g * GS:(g + 1) * GS] = 1.0 / GS

    xf = x.reshape(B, N_TOK, C)
    in_maps = []
    for core in range(8):
        b, h = divmod(core, 2)
        own = xf[b, h * NQ:(h + 1) * NQ]          # [NQ, C] fp32
        other = xf[b, (1 - h) * NQ:(2 - h) * NQ]
        perm = np.concatenate([own, other], axis=0)        # own half first
        xt = np.ascontiguousarray(perm.T.astype(ml_dtypes.bfloat16))  # [C, N]
        xr = np.ascontiguousarray(own + bs["bo"][None, :])  # residual (+bo)
        in_maps.append({
            "xt": xt,
            "xr": xr,
            "wq": ws["wq"], "wk": ws["wk"], "wv": ws["wv"], "wo": ws["wo"],
            "bq": bs["bq"], "bk": bs["bk"], "bv": bs["bv"],
            "gsc": gn_scale, "gbi": gn_bias,
            "gmat": gmat,
        })

    nc = _get_nc()
    res = run_bass_kernel_spmd(nc, in_maps, core_ids=list(range(8)))

    out = np.empty((B, N_TOK, C), np.float32)
    for core in range(8):
        b, h = divmod(core, 2)
        out[b, h * NQ:(h + 1) * NQ] = res.results[core]["out"]
    return out.reshape(B, H, W, C)


# revision 24
# speedup vs baseline: 1.2192x; 1.0052x over previous
"""AttnBlock (GroupNorm + single-head self-attention + residual) on 8 TRN2
NeuronCores.

Reference computation (per image b of 4, tokens N=64*64=4096, C=512):
    hn  = GroupNorm(x)  (32 groups, eps 1e-6, affine)
    q,k,v = hn @ wq + bq, ...
    attn = softmax(q @ k.T / sqrt(C)); out = attn @ v
    y   = x + out @ wo + bo

Sharding: one NeuronCore per (image, half): core 2b+h computes attention
rows [h*2048, (h+1)*2048) of image b. Each core redundantly computes
GroupNorm stats and full-image K/V (cheap vs. cross-core collectives) and
its own 2048 query rows. No inter-core communication.

Per-core layout: everything feature-major ([C, tokens]) so every matmul
contraction sits on the partition axis; the final projection naturally
returns to row-major. The host pre-transposes/casts x to bf16
feature-major per core (shard prep) and passes the residual rows in f32.

Device pipeline:
  1. GroupNorm statistics via bn_stats/bn_aggr on the raw bf16 x
     (feature-major; per-channel over tokens, then group-combined with a
     block-diagonal averaging matmul).
  2. The normalization hn = x*A + B is FOLDED INTO THE QKV WEIGHTS:
     W' = A∘W (row scale), b' = b + B@W. The projections then consume the
     raw x tiles directly - no normalize pass on the critical path.
  3. Attention: scoresT = kT^T q (feature-major both sides), exp on the
     Scalar engine without max subtraction (scores provably in [-2,2] for
     unit-normalized inputs), softmax denominator accumulated on the
     Vector engine, attn@v and output projection on TensorE, with the
     1/denominator applied per query row in the epilogue (softmax
     normalization commutes with the linear attn@v and output proj).
Compute dtype: bf16 operands, f32 PSUM accumulation.
"""

import sys

if "/opt/trn_rl_repo" not in sys.path:
    sys.path.insert(0, "/opt/trn_rl_repo")

import numpy as np
import ml_dtypes

import concourse.bass as bass
import concourse.tile as tile
from concourse import bacc, mybir
from concourse.bass_utils import run_bass_kernel_spmd

F32 = mybir.dt.float32
BF16 = mybir.dt.bfloat16

B, H, W, C = 4, 64, 64, 512
N_TOK = H * W            # tokens per image
NQ = N_TOK // 2          # query rows per core
G = 32                   # groups
GS = C // G              # channels per group (16)
EPS = 1e-6
SCALE = float(C) ** -0.5
CT = C // 128            # channel tiles (4)
JT = N_TOK // 128        # token tiles (32)
IB = NQ // 512           # query i-blocks (4)

_CACHE = {}


def _build():
    nc = bacc.Bacc("TRN2", target_bir_lowering=False)

    xt_e = nc.dram_tensor("xt", [C, N_TOK], BF16, kind="ExternalInput")
    xr_e = nc.dram_tensor("xr", [NQ, C], F32, kind="ExternalInput")
    w_e = {
        n: nc.dram_tensor(n, [C, C], BF16, kind="ExternalInput")
        for n in ("wq", "wk", "wv", "wo")
    }
    b_e = {
        n: nc.dram_tensor(n, [C], F32, kind="ExternalInput")
        for n in ("bq", "bk", "bv")
    }
    gs_e = nc.dram_tensor("gsc", [C], F32, kind="ExternalInput")
    gb_e = nc.dram_tensor("gbi", [C], F32, kind="ExternalInput")
    gm_e = nc.dram_tensor("gmat", [128, 128], F32, kind="ExternalInput")
    out_e = nc.dram_tensor("out", [NQ, C], F32, kind="ExternalOutput")

    def col(e):  # [C] dram -> [C,1] view for partition-major loads
        return e.ap().rearrange("(a b) -> a b", b=1)

    with tile.TileContext(nc) as tc:
        with (
            tc.tile_pool(name="const", bufs=1) as const,
            tc.tile_pool(name="big", bufs=1) as big,
            tc.tile_pool(name="stat", bufs=1) as stat,
            tc.tile_pool(name="ework", bufs=4) as ework,
            tc.tile_pool(name="attw", bufs=8) as attw,
            tc.tile_pool(name="owork", bufs=4) as owork,
            tc.tile_pool(name="xrw", bufs=3) as xrw,
            tc.tile_pool(name="rdenw", bufs=8) as rdenw,
        ):
            # ---- DMA issue order = need order: the per-queue completion
            # counters are cumulative, so anything queued behind a large
            # transfer waits for it. gmat/biases gate the first PE op;
            # x gates stats; weights are needed only ~20us in.
            gm_sb = const.tile([128, 128], F32, tag="gmat")
            nc.sync.dma_start(out=gm_sb, in_=gm_e.ap())

            # ---- x feature-major (bf16) ----
            # half-tile contiguous DMAs (512 KB each)
            xT = []
            for k in range(CT):
                t = big.tile([128, N_TOK], BF16, tag=f"xT{k}", name=f"xT{k}")
                for hh in range(2):
                    nc.sync.dma_start(
                        out=t[:, hh * 2048:(hh + 1) * 2048],
                        in_=xt_e.ap()[k * 128:(k + 1) * 128,
                                      hh * 2048:(hh + 1) * 2048],
                    )
                xT.append(t)

            bsb = {}
            for n in ("bq", "bk"):
                bsb[n] = []
                for m in range(CT):
                    t = const.tile([128, 1], F32, tag=f"b_{n}_{m}")
                    nc.sync.dma_start(out=t, in_=col(b_e[n])[m * 128:(m + 1) * 128, :])
                    bsb[n].append(t)
            bvb = const.tile([128, C], F32, tag="bvb")
            nc.sync.dma_start(
                out=bvb,
                in_=bass.AP(tensor=b_e["bv"], offset=0, ap=[[0, 128], [1, C]]),
            )
            gssb, gbsb = [], []
            for m in range(CT):
                t = const.tile([128, 1], F32, tag=f"gs_{m}")
                nc.sync.dma_start(out=t, in_=col(gs_e)[m * 128:(m + 1) * 128, :])
                gssb.append(t)
                t = const.tile([128, 1], F32, tag=f"gb_{m}")
                nc.sync.dma_start(out=t, in_=col(gb_e)[m * 128:(m + 1) * 128, :])
                gbsb.append(t)
            # ---- weights (largest, least urgent) ----
            wsb = {}
            for n in ("wq", "wk", "wv", "wo"):
                wsb[n] = []
                for k in range(CT):
                    t = const.tile([128, C], BF16, tag=f"w_{n}_{k}", name=f"w_{n}_{k}")
                    nc.sync.dma_start(out=t, in_=w_e[n].ap()[k * 128:(k + 1) * 128, :])
                    wsb[n].append(t)
            ones_bcol = const.tile([1, 128], F32, tag="ones_bcol")
            nc.vector.memset(ones_bcol, 1.0)
            onef = const.tile([1, 1], F32, tag="onef")
            nc.vector.memset(onef, 1.0)
            onesf_col = const.tile([128, 1], F32, tag="onesf_col")
            nc.vector.memset(onesf_col, 1.0)
            epst = const.tile([128, 1], F32, tag="epst")
            nc.vector.memset(epst, EPS)

            # ---- GroupNorm stats + weight folding + projections ----
            # ps_misc (4 tags x 1 buf = 4 banks) and ps_proj (4 banks) are
            # OPEN SIMULTANEOUSLY: a stacked open/close would make the
            # projection pool's banks WAR-depend on the whole stats phase.
            with (
                tc.tile_pool(name="ps_misc", bufs=1, space="PSUM") as psm,
                tc.tile_pool(name="ps_proj", bufs=4, space="PSUM") as psp,
            ):
                Af, Bbf = [], []   # A (f32 [128,1]); B cast to bf16 for matmuls
                # per-channel (mean, E[x^2]) per tile, split across engines:
                # tiles 0,1 on ACT (Copy/Square with accum_out; the 1/N and
                # 1/N per-element scales fold into the activation scale),
                # tiles 2,3 on DVE (bn_stats) - halves the serial stats chain.
                scr0 = stat.tile([128, N_TOK], BF16, tag="scr", name="scr0")
                scr = [scr0, scr0]
                sms = []
                for k in range(CT):
                    sm = stat.tile([128, 2], F32, tag=f"sm{k}", name=f"sm{k}")
                    if k < 2:
                        nc.scalar.activation(
                            out=scr[k], in_=xT[k],
                            func=mybir.ActivationFunctionType.Copy,
                            scale=1.0 / N_TOK, accum_out=sm[:, 0:1],
                        )
                        nc.scalar.activation(
                            out=scr[k], in_=xT[k],
                            func=mybir.ActivationFunctionType.Square,
                            scale=1.0 / float(np.sqrt(N_TOK)),
                            accum_out=sm[:, 1:2],
                        )
                    else:
                        stats = stat.tile([128, 8, 6], F32, tag=f"st{k}",
                                          name=f"st{k}")
                        for ch in range(8):
                            nc.vector.bn_stats(
                                out=stats[:, ch, :],
                                in_=xT[k][:, ch * 512:(ch + 1) * 512],
                            )
                        mv = stat.tile([128, 2], F32, tag=f"mv{k}", name=f"mv{k}")
                        nc.vector.bn_aggr(out=mv, in_=stats)
                        nc.vector.tensor_copy(out=sm[:, 0:1], in_=mv[:, 0:1])
                        nc.vector.tensor_mul(out=sm[:, 1:2], in0=mv[:, 0:1],
                                             in1=mv[:, 0:1])
                        nc.vector.tensor_add(out=sm[:, 1:2], in0=sm[:, 1:2],
                                             in1=mv[:, 1:2])
                    sms.append(sm)
                    # group-average via block-diagonal (1/GS) matrix
                    gps = psm.tile([128, 2], F32, tag="gps")
                    nc.tensor.matmul(gps, gm_sb, sm, start=True, stop=True)
                    gsb = stat.tile([128, 2], F32, tag=f"gsb{k}")
                    nc.vector.tensor_copy(out=gsb, in_=gps)
                    # var_g = E_g[x^2]-mean_g^2; A = rstd*scale; B = bias-mean_g*A
                    msq = stat.tile([128, 1], F32, tag=f"msq{k}")
                    nc.vector.tensor_mul(out=msq, in0=gsb[:, 0:1], in1=gsb[:, 0:1])
                    varg = stat.tile([128, 1], F32, tag=f"vg{k}")
                    nc.vector.tensor_sub(out=varg, in0=gsb[:, 1:2], in1=msq)
                    sd = stat.tile([128, 1], F32, tag=f"sd{k}")
                    nc.scalar.activation(
                        out=sd, in_=varg,
                        func=mybir.ActivationFunctionType.Sqrt,
                        bias=epst, scale=1.0,
                    )
                    rstd = stat.tile([128, 1], F32, tag=f"rs{k}")
                    nc.vector.reciprocal(out=rstd, in_=sd)
                    At = stat.tile([128, 1], F32, tag=f"A{k}")
                    nc.vector.tensor_mul(out=At, in0=rstd, in1=gssb[k])
                    mA = stat.tile([128, 1], F32, tag=f"mA{k}")
                    nc.vector.tensor_mul(out=mA, in0=gsb[:, 0:1], in1=At)
                    Bt = stat.tile([128, 1], F32, tag=f"B{k}")
                    nc.vector.tensor_sub(out=Bt, in0=gbsb[k], in1=mA)
                    Bb = stat.tile([128, 1], BF16, tag=f"Bb{k}")
                    nc.vector.tensor_copy(out=Bb, in_=Bt)
                    Af.append(At)
                    Bbf.append(Bb)

                # fold normalization into weights:
                #   W' = A (row) ∘ W ;  b' = b + B @ W
                # B@W matmuls read the ORIGINAL W (Tile orders them before the
                # in-place row scale below via WAR deps).
                badj = {}
                for n in ("wq", "wk", "wv"):
                    pb = psm.tile([1, 512], F32, tag="pb", name=f"pb_{n}")
                    for k in range(CT):
                        nc.tensor.matmul(
                            pb, Bbf[k], wsb[n][k],
                            start=(k == 0), stop=(k == CT - 1),
                        )
                    bs_ = stat.tile([1, 512], F32, tag=f"badj_{n}")
                    nc.vector.tensor_copy(out=bs_, in_=pb)
                    badj[n] = bs_
                # row-scale the weights on the Scalar engine (idle here) into
                # SEPARATE tiles: an in-place fold would WAR-depend on the
                # b-adjust matmuls above (which need the last tile's stats),
                # serializing the whole prologue. Separate outputs let the
                # k-th fold fire as soon as A[k] is ready, so QKV matmuls
                # start ~25us earlier.
                wf = {}
                for n in ("wq", "wk", "wv"):
                    wf[n] = []
                    for k in range(CT):
                        t = const.tile([128, C], BF16, tag=f"wf_{n}_{k}",
                                       name=f"wf_{n}_{k}")
                        nc.scalar.activation(
                            out=t, in_=wsb[n][k],
                            func=mybir.ActivationFunctionType.Copy,
                            scale=Af[k],
                        )
                        wf[n].append(t)
                wf["wo"] = wsb["wo"]
                # transpose b' pieces to per-partition layout for q/k;
                # build broadcast bias for v.
                bqf, bkf = [], []
                for n, dst in (("wq", bqf), ("wk", bkf)):
                    for m in range(CT):
                        pt = psm.tile([128, 1], F32, tag="pt", name=f"pt_{n}{m}")
                        nc.tensor.matmul(
                            pt, badj[n][0:1, m * 128:(m + 1) * 128], onef,
                            start=True, stop=True,
                        )
                        bf = stat.tile([128, 1], F32, tag=f"bf_{n}{m}")
                        base = bsb["bq"][m] if n == "wq" else bsb["bk"][m]
                        nc.vector.tensor_add(out=bf, in0=pt, in1=base)
                        dst.append(bf)
                pvb = psm.tile([128, 512], F32, tag="pvb")
                nc.tensor.matmul(pvb, ones_bcol, badj["wv"], start=True, stop=True)
                nc.vector.tensor_add(out=bvb, in0=pvb, in1=bvb)

                # ---- projections (raw x in, folded weights) ----
                kT = [big.tile([128, N_TOK], BF16, tag=f"kT{m}", name=f"kT{m}")
                      for m in range(CT)]
                qT = [big.tile([128, NQ], BF16, tag=f"qT{m}", name=f"qT{m}")
                      for m in range(CT)]
                v_sb = big.tile([128, JT, C], BF16, tag="v")
                for m in range(CT):
                    for nt in range(N_TOK // 512):
                        pk = psp.tile([128, 512], F32, tag="p")
                        for k in range(CT):
                            nc.tensor.matmul(
                                pk,
                                wf["wk"][k][:, m * 128:(m + 1) * 128],
                                xT[k][:, nt * 512:(nt + 1) * 512],
                                start=(k == 0), stop=(k == CT - 1),
                            )
                        nc.vector.tensor_scalar_add(
                            out=kT[m][:, nt * 512:(nt + 1) * 512],
                            in0=pk, scalar1=bkf[m],
                        )
                    for nt in range(NQ // 512):
                        pq = psp.tile([128, 512], F32, tag="p")
                        for k in range(CT):
                            nc.tensor.matmul(
                                pq,
                                wf["wq"][k][:, m * 128:(m + 1) * 128],
                                xT[k][:, nt * 512:(nt + 1) * 512],
                                start=(k == 0), stop=(k == CT - 1),
                            )
                        nc.vector.tensor_scalar_add(
                            out=qT[m][:, nt * 512:(nt + 1) * 512],
                            in0=pq, scalar1=bqf[m],
                        )
                for jt in range(JT):
                    pv = psp.tile([128, 512], F32, tag="p")
                    for k in range(CT):
                        nc.tensor.matmul(
                            pv,
                            xT[k][:, jt * 128:(jt + 1) * 128],
                            wf["wv"][k],
                            start=(k == 0), stop=(k == CT - 1),
                        )
                    nc.vector.tensor_add(out=v_sb[:, jt, :], in0=pv, in1=bvb)

            # ---- attention ----
            with (
                tc.tile_pool(name="ps_att", bufs=4, space="PSUM") as psa,
                tc.tile_pool(name="ps_s", bufs=2, space="PSUM") as pss,
                tc.tile_pool(name="ps_o", bufs=1, space="PSUM") as pso,
                tc.tile_pool(name="ps_den", bufs=1, space="PSUM") as psd,
            ):
                for ib in range(IB):
                    att_ps = [psa.tile([128, 512], F32, tag="att", name=f"att_ps{cs}")
                              for cs in range(CT)]
                    dacc = owork.tile([128, 512], F32, tag="dacc")
                    nc.vector.memset(dacc, 0.0)
                    for jt in range(JT):
                        s_ps = pss.tile([128, 512], F32, tag="s")
                        for k in range(CT):
                            nc.tensor.matmul(
                                s_ps,
                                kT[k][:, jt * 128:(jt + 1) * 128],
                                qT[k][:, ib * 512:(ib + 1) * 512],
                                start=(k == 0), stop=(k == CT - 1),
                            )
                        e_t = ework.tile([128, 512], BF16, tag="e")
                        nc.scalar.activation(
                            out=e_t, in_=s_ps,
                            func=mybir.ActivationFunctionType.Exp,
                            scale=SCALE,
                        )
                        for cs in range(CT):
                            nc.tensor.matmul(
                                att_ps[cs],
                                v_sb[:, jt, cs * 128:(cs + 1) * 128],
                                e_t,
                                start=(jt == 0), stop=(jt == JT - 1),
                            )
                        nc.vector.tensor_add(out=dacc, in0=dacc, in1=e_t)
                    # denominator: column sums of dacc (over j partitions)
                    den_ps = psd.tile([1, 512], F32, tag="den")
                    nc.tensor.matmul(den_ps, onesf_col, dacc, start=True, stop=True)
                    attT = []
                    for cs in range(CT):
                        t = attw.tile([128, 512], BF16, tag="attT", name=f"attT{cs}")
                        nc.vector.tensor_copy(out=t, in_=att_ps[cs])
                        attT.append(t)
                    den_sb = owork.tile([1, 512], F32, tag="den_sb")
                    nc.vector.tensor_copy(out=den_sb, in_=den_ps)
                    for it in range(4):
                        row0 = (ib * 4 + it) * 128
                        dT = pso.tile([128, 1], F32, tag="o",
                                      padded_shape=[128, 512], name=f"dT{it}")
                        nc.tensor.matmul(
                            dT, den_sb[0:1, it * 128:(it + 1) * 128], onef,
                            start=True, stop=True,
                        )
                        rden = rdenw.tile([128, 1], F32, tag="rden")
                        nc.vector.reciprocal(out=rden, in_=dT)
                        o_ps = pso.tile([128, 512], F32, tag="o", name=f"o_ps{it}")
                        for cs in range(CT):
                            nc.tensor.matmul(
                                o_ps,
                                attT[cs][:, it * 128:(it + 1) * 128],
                                wsb["wo"][cs],
                                start=(cs == 0), stop=(cs == CT - 1),
                            )
                        xr_t = xrw.tile([128, C], F32, tag="xr")
                        nc.sync.dma_start(
                            out=xr_t, in_=xr_e.ap()[row0:row0 + 128, :]
                        )
                        o_t = owork.tile([128, C], F32, tag="o")
                        nc.vector.scalar_tensor_tensor(
                            out=o_t, in0=o_ps, scalar=rden, in1=xr_t,
                            op0=mybir.AluOpType.mult, op1=mybir.AluOpType.add,
                        )
                        nc.sync.dma_start(
                            out=out_e.ap()[row0:row0 + 128, :], in_=o_t
                        )

    nc.compile()
    return nc


def _get_nc():
    if "nc" not in _CACHE:
        _CACHE["nc"] = _build()
    return _CACHE["nc"]


def kernel(**inputs) -> np.ndarray:
    x = np.asarray(inputs["x"], dtype=np.float32)          # [B,H,W,C]
    gn_scale = np.asarray(inputs["gn_scale"], np.float32)
    gn_bias = np.asarray(inputs["gn_bias"], np.float32)
    ws = {n: np.ascontiguousarray(
        np.asarray(inputs[n], np.float32).astype(ml_dtypes.bfloat16))
        for n in ("wq", "wk", "wv", "wo")}
    bs = {n: np.asarray(inputs[n], np.float32) for n in ("bq", "bk", "bv", "bo")}

    gmat = np.zeros((128, 128), np.float32)
    for g in range(128 // GS):
        gmat[g * GS:(g + 1) * GS, g * GS:(g + 1) * GS] = 1.0 / GS

    xf = x.reshape(B, N_TOK, C)
    in_maps = []
    for core in range(8):
        b, h = divmod(core, 2)
        own = xf[b, h * NQ:(h + 1) * NQ]          # [NQ, C] fp32
        other = xf[b, (1 - h) * NQ:(2 - h) * NQ]
        perm = np.concatenate([own, other], axis=0)        # own half first
        xt = np.ascontiguousarray(perm.T.astype(ml_dtypes.bfloat16))  # [C, N]
        xr = np.ascontiguousarray(own + bs["bo"][None, :])  # residual (+bo)
        in_maps.append({
            "xt": xt,
            "xr": xr,
            "wq": ws["wq"], "wk": ws["wk"], "wv": ws["wv"], "wo": ws["wo"],
            "bq": bs["bq"], "bk": bs["bk"], "bv": bs["bv"],
            "gsc": gn_scale, "gbi": gn_bias,
            "gmat": gmat,
        })

    nc = _get_nc()
    res = run_bass_kernel_spmd(nc, in_maps, core_ids=list(range(8)))

    out = np.empty((B, N_TOK, C), np.float32)
    for core in range(8):
        b, h = divmod(core, 2)
        out[b, h * NQ:(h + 1) * NQ] = res.results[core]["out"]
    return out.reshape(B, H, W, C)


# revision 25
# speedup vs baseline: 1.5688x; 1.2867x over previous
"""AttnBlock (GroupNorm + single-head self-attention + residual) on 8 TRN2
NeuronCores.

Reference computation (per image b of 4, tokens N=64*64=4096, C=512):
    hn  = GroupNorm(x)  (32 groups, eps 1e-6, affine)
    q,k,v = hn @ wq + bq, ...
    attn = softmax(q @ k.T / sqrt(C)); out = attn @ v
    y   = x + out @ wo + bo

Sharding: one NeuronCore per (image, half): core 2b+h computes attention
rows [h*2048, (h+1)*2048) of image b. Each core redundantly computes
GroupNorm stats and full-image K/V (cheap vs. cross-core collectives) and
its own 2048 query rows. No inter-core communication.

Per-core layout: everything feature-major ([C, tokens]) so every matmul
contraction sits on the partition axis; the final projection naturally
returns to row-major. The host pre-transposes/casts x to bf16
feature-major per core (shard prep) and passes the residual rows in f32.

Device pipeline:
  1. GroupNorm statistics via bn_stats/bn_aggr on the raw bf16 x
     (feature-major; per-channel over tokens, then group-combined with a
     block-diagonal averaging matmul).
  2. The normalization hn = x*A + B is FOLDED INTO THE QKV WEIGHTS:
     W' = A∘W (row scale), b' = b + B@W. The projections then consume the
     raw x tiles directly - no normalize pass on the critical path.
  3. Attention: scoresT = kT^T q (feature-major both sides), exp on the
     Scalar engine without max subtraction (scores provably in [-2,2] for
     unit-normalized inputs), softmax denominator accumulated on the
     Vector engine, attn@v and output projection on TensorE, with the
     1/denominator applied per query row in the epilogue (softmax
     normalization commutes with the linear attn@v and output proj).
Compute dtype: bf16 operands, f32 PSUM accumulation.
"""

import sys

if "/opt/trn_rl_repo" not in sys.path:
    sys.path.insert(0, "/opt/trn_rl_repo")

import numpy as np
import ml_dtypes

import concourse.bass as bass
import concourse.tile as tile
from concourse import bacc, mybir
from concourse.bass_utils import run_bass_kernel_spmd

F32 = mybir.dt.float32
BF16 = mybir.dt.bfloat16
FP8 = mybir.dt.float8e4

B, H, W, C = 4, 64, 64, 512
N_TOK = H * W            # tokens per image
NQ = N_TOK // 2          # query rows per core
G = 32                   # groups
GS = C // G              # channels per group (16)
EPS = 1e-6
SCALE = float(C) ** -0.5
CT = C // 128            # channel tiles (4)
JT = N_TOK // 128        # token tiles (32)
IB = NQ // 512           # query i-blocks (4)

_CACHE = {}


def _build():
    nc = bacc.Bacc("TRN2", target_bir_lowering=False)

    xt_e = nc.dram_tensor("xt", [C, N_TOK], BF16, kind="ExternalInput")
    xr_e = nc.dram_tensor("xr", [NQ, C], F32, kind="ExternalInput")
    w_e = {
        n: nc.dram_tensor(n, [C, C], BF16, kind="ExternalInput")
        for n in ("wq", "wk", "wv", "wo")
    }
    b_e = {
        n: nc.dram_tensor(n, [C], F32, kind="ExternalInput")
        for n in ("bq", "bk", "bv")
    }
    gs_e = nc.dram_tensor("gsc", [C], F32, kind="ExternalInput")
    gb_e = nc.dram_tensor("gbi", [C], F32, kind="ExternalInput")
    gm_e = nc.dram_tensor("gmat", [128, 128], F32, kind="ExternalInput")
    out_e = nc.dram_tensor("out", [NQ, C], F32, kind="ExternalOutput")

    def col(e):  # [C] dram -> [C,1] view for partition-major loads
        return e.ap().rearrange("(a b) -> a b", b=1)

    with tile.TileContext(nc) as tc:
        with (
            tc.tile_pool(name="const", bufs=1) as const,
            tc.tile_pool(name="big", bufs=1) as big,
            tc.tile_pool(name="stat", bufs=1) as stat,
            tc.tile_pool(name="ework", bufs=4) as ework,
            tc.tile_pool(name="attw", bufs=8) as attw,
            tc.tile_pool(name="owork", bufs=4) as owork,
            tc.tile_pool(name="xrw", bufs=3) as xrw,
            tc.tile_pool(name="rdenw", bufs=8) as rdenw,
        ):
            # ---- DMA issue order = need order: the per-queue completion
            # counters are cumulative, so anything queued behind a large
            # transfer waits for it. gmat/biases gate the first PE op;
            # x gates stats; weights are needed only ~20us in.
            gm_sb = const.tile([128, 128], F32, tag="gmat")
            nc.sync.dma_start(out=gm_sb, in_=gm_e.ap())

            # ---- x feature-major (bf16) ----
            # half-tile contiguous DMAs (512 KB each)
            xT = []
            for k in range(CT):
                t = big.tile([128, N_TOK], BF16, tag=f"xT{k}", name=f"xT{k}")
                for hh in range(2):
                    nc.sync.dma_start(
                        out=t[:, hh * 2048:(hh + 1) * 2048],
                        in_=xt_e.ap()[k * 128:(k + 1) * 128,
                                      hh * 2048:(hh + 1) * 2048],
                    )
                xT.append(t)

            bsb = {}
            for n in ("bq", "bk"):
                bsb[n] = []
                for m in range(CT):
                    t = const.tile([128, 1], F32, tag=f"b_{n}_{m}")
                    nc.sync.dma_start(out=t, in_=col(b_e[n])[m * 128:(m + 1) * 128, :])
                    bsb[n].append(t)
            bvb = const.tile([128, C], F32, tag="bvb")
            nc.sync.dma_start(
                out=bvb,
                in_=bass.AP(tensor=b_e["bv"], offset=0, ap=[[0, 128], [1, C]]),
            )
            gssb, gbsb = [], []
            for m in range(CT):
                t = const.tile([128, 1], F32, tag=f"gs_{m}")
                nc.sync.dma_start(out=t, in_=col(gs_e)[m * 128:(m + 1) * 128, :])
                gssb.append(t)
                t = const.tile([128, 1], F32, tag=f"gb_{m}")
                nc.sync.dma_start(out=t, in_=col(gb_e)[m * 128:(m + 1) * 128, :])
                gbsb.append(t)
            # ---- weights (largest, least urgent) ----
            wsb = {}
            for n in ("wq", "wk", "wv", "wo"):
                wsb[n] = []
                for k in range(CT):
                    t = const.tile([128, C], BF16, tag=f"w_{n}_{k}", name=f"w_{n}_{k}")
                    nc.sync.dma_start(out=t, in_=w_e[n].ap()[k * 128:(k + 1) * 128, :])
                    wsb[n].append(t)
            ones_bcol = const.tile([1, 128], F32, tag="ones_bcol")
            nc.vector.memset(ones_bcol, 1.0)
            onef = const.tile([1, 1], F32, tag="onef")
            nc.vector.memset(onef, 1.0)
            onesf_col = const.tile([128, 1], F32, tag="onesf_col")
            nc.vector.memset(onesf_col, 1.0)
            epst = const.tile([128, 1], F32, tag="epst")
            nc.vector.memset(epst, EPS)

            # ---- GroupNorm stats + weight folding + projections ----
            # ps_misc (4 tags x 1 buf = 4 banks) and ps_proj (4 banks) are
            # OPEN SIMULTANEOUSLY: a stacked open/close would make the
            # projection pool's banks WAR-depend on the whole stats phase.
            with (
                tc.tile_pool(name="ps_misc", bufs=1, space="PSUM") as psm,
                tc.tile_pool(name="ps_proj", bufs=4, space="PSUM") as psp,
            ):
                Af, Bbf = [], []   # A (f32 [128,1]); B cast to bf16 for matmuls
                # per-channel (mean, E[x^2]) per tile, split across engines:
                # tiles 0,1 on ACT (Copy/Square with accum_out; the 1/N and
                # 1/N per-element scales fold into the activation scale),
                # tiles 2,3 on DVE (bn_stats) - halves the serial stats chain.
                scr0 = stat.tile([128, N_TOK], BF16, tag="scr", name="scr0")
                scr = [scr0, scr0]
                sms = []
                for k in range(CT):
                    sm = stat.tile([128, 2], F32, tag=f"sm{k}", name=f"sm{k}")
                    if k < 2:
                        nc.scalar.activation(
                            out=scr[k], in_=xT[k],
                            func=mybir.ActivationFunctionType.Copy,
                            scale=1.0 / N_TOK, accum_out=sm[:, 0:1],
                        )
                        nc.scalar.activation(
                            out=scr[k], in_=xT[k],
                            func=mybir.ActivationFunctionType.Square,
                            scale=1.0 / float(np.sqrt(N_TOK)),
                            accum_out=sm[:, 1:2],
                        )
                    else:
                        stats = stat.tile([128, 8, 6], F32, tag=f"st{k}",
                                          name=f"st{k}")
                        for ch in range(8):
                            nc.vector.bn_stats(
                                out=stats[:, ch, :],
                                in_=xT[k][:, ch * 512:(ch + 1) * 512],
                            )
                        mv = stat.tile([128, 2], F32, tag=f"mv{k}", name=f"mv{k}")
                        nc.vector.bn_aggr(out=mv, in_=stats)
                        nc.vector.tensor_copy(out=sm[:, 0:1], in_=mv[:, 0:1])
                        nc.vector.tensor_mul(out=sm[:, 1:2], in0=mv[:, 0:1],
                                             in1=mv[:, 0:1])
                        nc.vector.tensor_add(out=sm[:, 1:2], in0=sm[:, 1:2],
                                             in1=mv[:, 1:2])
                    sms.append(sm)
                    # group-average via block-diagonal (1/GS) matrix
                    gps = psm.tile([128, 2], F32, tag="gps")
                    nc.tensor.matmul(gps, gm_sb, sm, start=True, stop=True)
                    gsb = stat.tile([128, 2], F32, tag=f"gsb{k}")
                    nc.vector.tensor_copy(out=gsb, in_=gps)
                    # var_g = E_g[x^2]-mean_g^2; A = rstd*scale; B = bias-mean_g*A
                    msq = stat.tile([128, 1], F32, tag=f"msq{k}")
                    nc.vector.tensor_mul(out=msq, in0=gsb[:, 0:1], in1=gsb[:, 0:1])
                    varg = stat.tile([128, 1], F32, tag=f"vg{k}")
                    nc.vector.tensor_sub(out=varg, in0=gsb[:, 1:2], in1=msq)
                    sd = stat.tile([128, 1], F32, tag=f"sd{k}")
                    nc.scalar.activation(
                        out=sd, in_=varg,
                        func=mybir.ActivationFunctionType.Sqrt,
                        bias=epst, scale=1.0,
                    )
                    rstd = stat.tile([128, 1], F32, tag=f"rs{k}")
                    nc.vector.reciprocal(out=rstd, in_=sd)
                    At = stat.tile([128, 1], F32, tag=f"A{k}")
                    nc.vector.tensor_mul(out=At, in0=rstd, in1=gssb[k])
                    mA = stat.tile([128, 1], F32, tag=f"mA{k}")
                    nc.vector.tensor_mul(out=mA, in0=gsb[:, 0:1], in1=At)
                    Bt = stat.tile([128, 1], F32, tag=f"B{k}")
                    nc.vector.tensor_sub(out=Bt, in0=gbsb[k], in1=mA)
                    Bb = stat.tile([128, 1], BF16, tag=f"Bb{k}")
                    nc.vector.tensor_copy(out=Bb, in_=Bt)
                    Af.append(At)
                    Bbf.append(Bb)

                # fold normalization into weights:
                #   W' = A (row) ∘ W ;  b' = b + B @ W
                # B@W matmuls read the ORIGINAL W (Tile orders them before the
                # in-place row scale below via WAR deps).
                badj = {}
                for n in ("wq", "wk", "wv"):
                    pb = psm.tile([1, 512], F32, tag="pb", name=f"pb_{n}")
                    for k in range(CT):
                        nc.tensor.matmul(
                            pb, Bbf[k], wsb[n][k],
                            start=(k == 0), stop=(k == CT - 1),
                        )
                    bs_ = stat.tile([1, 512], F32, tag=f"badj_{n}")
                    nc.vector.tensor_copy(out=bs_, in_=pb)
                    badj[n] = bs_
                # row-scale the weights on the Scalar engine (idle here) into
                # SEPARATE tiles: an in-place fold would WAR-depend on the
                # b-adjust matmuls above (which need the last tile's stats),
                # serializing the whole prologue. Separate outputs let the
                # k-th fold fire as soon as A[k] is ready, so QKV matmuls
                # start ~25us earlier.
                wf = {}
                for n in ("wq", "wk", "wv"):
                    wf[n] = []
                    for k in range(CT):
                        t = const.tile([128, C], BF16, tag=f"wf_{n}_{k}",
                                       name=f"wf_{n}_{k}")
                        nc.scalar.activation(
                            out=t, in_=wsb[n][k],
                            func=mybir.ActivationFunctionType.Copy,
                            scale=Af[k],
                        )
                        wf[n].append(t)
                wf["wo"] = wsb["wo"]
                # transpose b' pieces to per-partition layout for q/k;
                # build broadcast bias for v.
                bqf, bkf = [], []
                for n, dst in (("wq", bqf), ("wk", bkf)):
                    for m in range(CT):
                        pt = psm.tile([128, 1], F32, tag="pt", name=f"pt_{n}{m}")
                        nc.tensor.matmul(
                            pt, badj[n][0:1, m * 128:(m + 1) * 128], onef,
                            start=True, stop=True,
                        )
                        bf = stat.tile([128, 1], F32, tag=f"bf_{n}{m}")
                        base = bsb["bq"][m] if n == "wq" else bsb["bk"][m]
                        nc.vector.tensor_add(out=bf, in0=pt, in1=base)
                        dst.append(bf)
                pvb = psm.tile([128, 512], F32, tag="pvb")
                nc.tensor.matmul(pvb, ones_bcol, badj["wv"], start=True, stop=True)
                nc.vector.tensor_add(out=bvb, in0=pvb, in1=bvb)

                # ---- projections (raw x in, folded weights) ----
                # q/k/v stored fp8-e4m3 in DoubleRow-interleaved layout
                # [128, ktile, free] (contraction index = ktile*128+partition)
                kT8 = big.tile([128, CT, N_TOK], FP8, tag="kT8")
                qT8 = big.tile([128, CT, NQ], FP8, tag="qT8")
                v_sb = big.tile([128, JT, C], FP8, tag="v")
                for m in range(CT):
                    for nt in range(N_TOK // 512):
                        pk = psp.tile([128, 512], F32, tag="p")
                        for k in range(CT):
                            nc.tensor.matmul(
                                pk,
                                wf["wk"][k][:, m * 128:(m + 1) * 128],
                                xT[k][:, nt * 512:(nt + 1) * 512],
                                start=(k == 0), stop=(k == CT - 1),
                            )
                        nc.vector.tensor_scalar_add(
                            out=kT8[:, m, nt * 512:(nt + 1) * 512],
                            in0=pk, scalar1=bkf[m],
                        )
                    for nt in range(NQ // 512):
                        pq = psp.tile([128, 512], F32, tag="p")
                        for k in range(CT):
                            nc.tensor.matmul(
                                pq,
                                wf["wq"][k][:, m * 128:(m + 1) * 128],
                                xT[k][:, nt * 512:(nt + 1) * 512],
                                start=(k == 0), stop=(k == CT - 1),
                            )
                        nc.vector.tensor_scalar_add(
                            out=qT8[:, m, nt * 512:(nt + 1) * 512],
                            in0=pq, scalar1=bqf[m],
                        )
                for jt in range(JT):
                    pv = psp.tile([128, 512], F32, tag="p")
                    for k in range(CT):
                        nc.tensor.matmul(
                            pv,
                            xT[k][:, jt * 128:(jt + 1) * 128],
                            wf["wv"][k],
                            start=(k == 0), stop=(k == CT - 1),
                        )
                    nc.vector.tensor_add(out=v_sb[:, jt, :], in0=pv, in1=bvb)

            # ---- attention ----
            with (
                tc.tile_pool(name="ps_att", bufs=4, space="PSUM") as psa,
                tc.tile_pool(name="ps_s", bufs=2, space="PSUM") as pss,
                tc.tile_pool(name="ps_o", bufs=1, space="PSUM") as pso,
                tc.tile_pool(name="ps_den", bufs=1, space="PSUM") as psd,
            ):
                NP_ = JT // 2  # j-tile pairs per i-block
                for ib in range(IB):
                    att_ps = [psa.tile([128, 512], F32, tag="att", name=f"att_ps{cs}")
                              for cs in range(CT)]
                    dacc = owork.tile([128, 512], F32, tag="dacc")
                    nc.vector.memset(dacc, 0.0)
                    qs = qT8[:, :, ib * 512:(ib + 1) * 512]
                    for g in range(NP_):
                        e_p = ework.tile([128, 2, 512], FP8, tag="e")
                        for o in range(2):
                            jt = 2 * g + o
                            s_ps = pss.tile([128, 512], F32, tag="s",
                                            name=f"s_ps{o}")
                            for kk in range(2):
                                nc.tensor.matmul(
                                    s_ps,
                                    kT8[:, 2 * kk:2 * kk + 2,
                                        jt * 128:(jt + 1) * 128],
                                    qs[:, 2 * kk:2 * kk + 2, :],
                                    start=(kk == 0), stop=(kk == 1),
                                    perf_mode=mybir.MatmulPerfMode.DoubleRow,
                                )
                            nc.scalar.activation(
                                out=e_p[:, o, :], in_=s_ps,
                                func=mybir.ActivationFunctionType.Exp,
                                scale=SCALE,
                            )
                            nc.vector.tensor_add(out=dacc, in0=dacc,
                                                 in1=e_p[:, o, :])
                        for cs in range(CT):
                            nc.tensor.matmul(
                                att_ps[cs],
                                v_sb[:, 2 * g:2 * g + 2,
                                     cs * 128:(cs + 1) * 128],
                                e_p,
                                start=(g == 0), stop=(g == NP_ - 1),
                                perf_mode=mybir.MatmulPerfMode.DoubleRow,
                            )
                    # denominator: column sums of dacc (over j partitions)
                    den_ps = psd.tile([1, 512], F32, tag="den")
                    nc.tensor.matmul(den_ps, onesf_col, dacc, start=True, stop=True)
                    attT = []
                    for cs in range(CT):
                        t = attw.tile([128, 512], BF16, tag="attT", name=f"attT{cs}")
                        nc.vector.tensor_copy(out=t, in_=att_ps[cs])
                        attT.append(t)
                    den_sb = owork.tile([1, 512], F32, tag="den_sb")
                    nc.vector.tensor_copy(out=den_sb, in_=den_ps)
                    for it in range(4):
                        row0 = (ib * 4 + it) * 128
                        dT = pso.tile([128, 1], F32, tag="o",
                                      padded_shape=[128, 512], name=f"dT{it}")
                        nc.tensor.matmul(
                            dT, den_sb[0:1, it * 128:(it + 1) * 128], onef,
                            start=True, stop=True,
                        )
                        rden = rdenw.tile([128, 1], F32, tag="rden")
                        nc.vector.reciprocal(out=rden, in_=dT)
                        o_ps = pso.tile([128, 512], F32, tag="o", name=f"o_ps{it}")
                        for cs in range(CT):
                            nc.tensor.matmul(
                                o_ps,
                                attT[cs][:, it * 128:(it + 1) * 128],
                                wsb["wo"][cs],
                                start=(cs == 0), stop=(cs == CT - 1),
                            )
                        xr_t = xrw.tile([128, C], F32, tag="xr")
                        nc.sync.dma_start(
                            out=xr_t, in_=xr_e.ap()[row0:row0 + 128, :]
                        )
                        o_t = owork.tile([128, C], F32, tag="o")
                        nc.vector.scalar_tensor_tensor(
                            out=o_t, in0=o_ps, scalar=rden, in1=xr_t,
                            op0=mybir.AluOpType.mult, op1=mybir.AluOpType.add,
                        )
                        nc.sync.dma_start(
                            out=out_e.ap()[row0:row0 + 128, :], in_=o_t
                        )

    nc.compile()
    return nc


def _get_nc():
    if "nc" not in _CACHE:
        _CACHE["nc"] = _build()
    return _CACHE["nc"]


def kernel(**inputs) -> np.ndarray:
    x = np.asarray(inputs["x"], dtype=np.float32)          # [B,H,W,C]
    gn_scale = np.asarray(inputs["gn_scale"], np.float32)
    gn_bias = np.asarray(inputs["gn_bias"], np.float32)
    ws = {n: np.ascontiguousarray(
        np.asarray(inputs[n], np.float32).astype(ml_dtypes.bfloat16))
        for n in ("wq", "wk", "wv", "wo")}
    bs = {n: np.asarray(inputs[n], np.float32) for n in ("bq", "bk", "bv", "bo")}

    gmat = np.zeros((128, 128), np.float32)
    for g in range(128 // GS):
        gmat[g * GS:(g + 1) * GS, g * GS:(g + 1) * GS] = 1.0 / GS

    xf = x.reshape(B, N_TOK, C)
    in_maps = []
    for core in range(8):
        b, h = divmod(core, 2)
        own = xf[b, h * NQ:(h + 1) * NQ]          # [NQ, C] fp32
        other = xf[b, (1 - h) * NQ:(2 - h) * NQ]
        perm = np.concatenate([own, other], axis=0)        # own half first
        xt = np.ascontiguousarray(perm.T.astype(ml_dtypes.bfloat16))  # [C, N]
        xr = np.ascontiguousarray(own + bs["bo"][None, :])  # residual (+bo)
        in_maps.append({
            "xt": xt,
            "xr": xr,
            "wq": ws["wq"], "wk": ws["wk"], "wv": ws["wv"], "wo": ws["wo"],
            "bq": bs["bq"], "bk": bs["bk"], "bv": bs["bv"],
            "gsc": gn_scale, "gbi": gn_bias,
            "gmat": gmat,
        })

    nc = _get_nc()
    res = run_bass_kernel_spmd(nc, in_maps, core_ids=list(range(8)))

    out = np.empty((B, N_TOK, C), np.float32)
    for core in range(8):
        b, h = divmod(core, 2)
        out[b, h * NQ:(h + 1) * NQ] = res.results[core]["out"]
    return out.reshape(B, H, W, C)


# revision 26
# speedup vs baseline: 1.6490x; 1.0512x over previous
"""AttnBlock (GroupNorm + single-head self-attention + residual) on 8 TRN2
NeuronCores.

Reference computation (per image b of 4, tokens N=64*64=4096, C=512):
    hn  = GroupNorm(x)  (32 groups, eps 1e-6, affine)
    q,k,v = hn @ wq + bq, ...
    attn = softmax(q @ k.T / sqrt(C)); out = attn @ v
    y   = x + out @ wo + bo

Sharding: one NeuronCore per (image, half): core 2b+h computes attention
rows [h*2048, (h+1)*2048) of image b. Each core redundantly computes
GroupNorm stats and full-image K/V (cheap vs. cross-core collectives) and
its own 2048 query rows. No inter-core communication.

Per-core layout: everything feature-major ([C, tokens]) so every matmul
contraction sits on the partition axis; the final projection naturally
returns to row-major. The host pre-transposes/casts x to bf16
feature-major per core (shard prep) and passes the residual rows in f32.

Device pipeline:
  1. GroupNorm statistics via bn_stats/bn_aggr on the raw bf16 x
     (feature-major; per-channel over tokens, then group-combined with a
     block-diagonal averaging matmul).
  2. The normalization hn = x*A + B is FOLDED INTO THE QKV WEIGHTS:
     W' = A∘W (row scale), b' = b + B@W. The projections then consume the
     raw x tiles directly - no normalize pass on the critical path.
  3. Attention: scoresT = kT^T q (feature-major both sides), exp on the
     Scalar engine without max subtraction (scores provably in [-2,2] for
     unit-normalized inputs), softmax denominator accumulated on the
     Vector engine, attn@v and output projection on TensorE, with the
     1/denominator applied per query row in the epilogue (softmax
     normalization commutes with the linear attn@v and output proj).
Compute dtype: bf16 operands, f32 PSUM accumulation.
"""

import sys

if "/opt/trn_rl_repo" not in sys.path:
    sys.path.insert(0, "/opt/trn_rl_repo")

import numpy as np
import ml_dtypes

import concourse.bass as bass
import concourse.tile as tile
from concourse import bacc, mybir
from concourse.bass_utils import run_bass_kernel_spmd

F32 = mybir.dt.float32
BF16 = mybir.dt.bfloat16
FP8 = mybir.dt.float8e4

B, H, W, C = 4, 64, 64, 512
N_TOK = H * W            # tokens per image
NQ = N_TOK // 2          # query rows per core
G = 32                   # groups
GS = C // G              # channels per group (16)
EPS = 1e-6
SCALE = float(C) ** -0.5
CT = C // 128            # channel tiles (4)
JT = N_TOK // 128        # token tiles (32)
IB = NQ // 512           # query i-blocks (4)

_CACHE = {}


def _build():
    nc = bacc.Bacc("TRN2", target_bir_lowering=False)

    xt_e = nc.dram_tensor("xt", [C, N_TOK], BF16, kind="ExternalInput")
    xr_e = nc.dram_tensor("xr", [NQ, C], F32, kind="ExternalInput")
    w_e = {
        n: nc.dram_tensor(n, [C, C], BF16, kind="ExternalInput")
        for n in ("wq", "wk", "wv", "wo")
    }
    b_e = {
        n: nc.dram_tensor(n, [C], F32, kind="ExternalInput")
        for n in ("bq", "bk", "bv")
    }
    gs_e = nc.dram_tensor("gsc", [C], F32, kind="ExternalInput")
    gb_e = nc.dram_tensor("gbi", [C], F32, kind="ExternalInput")
    gm_e = nc.dram_tensor("gmat", [128, 128], F32, kind="ExternalInput")
    out_e = nc.dram_tensor("out", [NQ, C], F32, kind="ExternalOutput")

    def col(e):  # [C] dram -> [C,1] view for partition-major loads
        return e.ap().rearrange("(a b) -> a b", b=1)

    with tile.TileContext(nc) as tc:
        with (
            tc.tile_pool(name="const", bufs=1) as const,
            tc.tile_pool(name="big", bufs=1) as big,
            tc.tile_pool(name="stat", bufs=1) as stat,
            tc.tile_pool(name="ework", bufs=6) as ework,
            tc.tile_pool(name="attw", bufs=8) as attw,
            tc.tile_pool(name="owork", bufs=4) as owork,
            tc.tile_pool(name="xrw", bufs=3) as xrw,
            tc.tile_pool(name="rdenw", bufs=8) as rdenw,
        ):
            # ---- DMA issue order = need order: the per-queue completion
            # counters are cumulative, so anything queued behind a large
            # transfer waits for it. gmat/biases gate the first PE op;
            # x gates stats; weights are needed only ~20us in.
            gm_sb = const.tile([128, 128], F32, tag="gmat")
            nc.sync.dma_start(out=gm_sb, in_=gm_e.ap())

            # ---- x feature-major (bf16) ----
            # half-tile contiguous DMAs (512 KB each)
            xT = []
            for k in range(CT):
                t = big.tile([128, N_TOK], BF16, tag=f"xT{k}", name=f"xT{k}")
                for hh in range(2):
                    nc.sync.dma_start(
                        out=t[:, hh * 2048:(hh + 1) * 2048],
                        in_=xt_e.ap()[k * 128:(k + 1) * 128,
                                      hh * 2048:(hh + 1) * 2048],
                    )
                xT.append(t)

            bsb = {}
            for n in ("bq", "bk"):
                bsb[n] = []
                for m in range(CT):
                    t = const.tile([128, 1], F32, tag=f"b_{n}_{m}")
                    nc.sync.dma_start(out=t, in_=col(b_e[n])[m * 128:(m + 1) * 128, :])
                    bsb[n].append(t)
            bvb = const.tile([128, C], F32, tag="bvb")
            nc.sync.dma_start(
                out=bvb,
                in_=bass.AP(tensor=b_e["bv"], offset=0, ap=[[0, 128], [1, C]]),
            )
            gssb, gbsb = [], []
            for m in range(CT):
                t = const.tile([128, 1], F32, tag=f"gs_{m}")
                nc.sync.dma_start(out=t, in_=col(gs_e)[m * 128:(m + 1) * 128, :])
                gssb.append(t)
                t = const.tile([128, 1], F32, tag=f"gb_{m}")
                nc.sync.dma_start(out=t, in_=col(gb_e)[m * 128:(m + 1) * 128, :])
                gbsb.append(t)
            # ---- weights (largest, least urgent) ----
            wsb = {}
            for n in ("wq", "wk", "wv", "wo"):
                wsb[n] = []
                for k in range(CT):
                    t = const.tile([128, C], BF16, tag=f"w_{n}_{k}", name=f"w_{n}_{k}")
                    nc.sync.dma_start(out=t, in_=w_e[n].ap()[k * 128:(k + 1) * 128, :])
                    wsb[n].append(t)
            ones_bcol = const.tile([1, 128], F32, tag="ones_bcol")
            nc.vector.memset(ones_bcol, 1.0)
            onef = const.tile([1, 1], F32, tag="onef")
            nc.vector.memset(onef, 1.0)
            onesf_col = const.tile([128, 1], F32, tag="onesf_col")
            nc.vector.memset(onesf_col, 1.0)
            epst = const.tile([128, 1], F32, tag="epst")
            nc.vector.memset(epst, EPS)

            # ---- GroupNorm stats + weight folding + projections ----
            # ps_misc (4 tags x 1 buf = 4 banks) and ps_proj (4 banks) are
            # OPEN SIMULTANEOUSLY: a stacked open/close would make the
            # projection pool's banks WAR-depend on the whole stats phase.
            with (
                tc.tile_pool(name="ps_misc", bufs=1, space="PSUM") as psm,
                tc.tile_pool(name="ps_proj", bufs=4, space="PSUM") as psp,
            ):
                Af, Bbf = [], []   # A (f32 [128,1]); B cast to bf16 for matmuls
                # per-channel (mean, E[x^2]) per tile, split across engines:
                # tiles 0,1 on ACT (Copy/Square with accum_out; the 1/N and
                # 1/N per-element scales fold into the activation scale),
                # tiles 2,3 on DVE (bn_stats) - halves the serial stats chain.
                scr0 = stat.tile([128, N_TOK], BF16, tag="scr", name="scr0")
                scr = [scr0, scr0]
                sms = []
                for k in range(CT):
                    sm = stat.tile([128, 2], F32, tag=f"sm{k}", name=f"sm{k}")
                    if k < 2:
                        nc.scalar.activation(
                            out=scr[k], in_=xT[k],
                            func=mybir.ActivationFunctionType.Copy,
                            scale=1.0 / N_TOK, accum_out=sm[:, 0:1],
                        )
                        nc.scalar.activation(
                            out=scr[k], in_=xT[k],
                            func=mybir.ActivationFunctionType.Square,
                            scale=1.0 / float(np.sqrt(N_TOK)),
                            accum_out=sm[:, 1:2],
                        )
                    else:
                        stats = stat.tile([128, 8, 6], F32, tag=f"st{k}",
                                          name=f"st{k}")
                        for ch in range(8):
                            nc.vector.bn_stats(
                                out=stats[:, ch, :],
                                in_=xT[k][:, ch * 512:(ch + 1) * 512],
                            )
                        mv = stat.tile([128, 2], F32, tag=f"mv{k}", name=f"mv{k}")
                        nc.vector.bn_aggr(out=mv, in_=stats)
                        nc.vector.tensor_copy(out=sm[:, 0:1], in_=mv[:, 0:1])
                        nc.vector.tensor_mul(out=sm[:, 1:2], in0=mv[:, 0:1],
                                             in1=mv[:, 0:1])
                        nc.vector.tensor_add(out=sm[:, 1:2], in0=sm[:, 1:2],
                                             in1=mv[:, 1:2])
                    sms.append(sm)
                    # group-average via block-diagonal (1/GS) matrix
                    gps = psm.tile([128, 2], F32, tag="gps")
                    nc.tensor.matmul(gps, gm_sb, sm, start=True, stop=True)
                    gsb = stat.tile([128, 2], F32, tag=f"gsb{k}")
                    nc.vector.tensor_copy(out=gsb, in_=gps)
                    # var_g = E_g[x^2]-mean_g^2; A = rstd*scale; B = bias-mean_g*A
                    msq = stat.tile([128, 1], F32, tag=f"msq{k}")
                    nc.vector.tensor_mul(out=msq, in0=gsb[:, 0:1], in1=gsb[:, 0:1])
                    varg = stat.tile([128, 1], F32, tag=f"vg{k}")
                    nc.vector.tensor_sub(out=varg, in0=gsb[:, 1:2], in1=msq)
                    sd = stat.tile([128, 1], F32, tag=f"sd{k}")
                    nc.scalar.activation(
                        out=sd, in_=varg,
                        func=mybir.ActivationFunctionType.Sqrt,
                        bias=epst, scale=1.0,
                    )
                    rstd = stat.tile([128, 1], F32, tag=f"rs{k}")
                    nc.vector.reciprocal(out=rstd, in_=sd)
                    At = stat.tile([128, 1], F32, tag=f"A{k}")
                    nc.vector.tensor_mul(out=At, in0=rstd, in1=gssb[k])
                    mA = stat.tile([128, 1], F32, tag=f"mA{k}")
                    nc.vector.tensor_mul(out=mA, in0=gsb[:, 0:1], in1=At)
                    Bt = stat.tile([128, 1], F32, tag=f"B{k}")
                    nc.vector.tensor_sub(out=Bt, in0=gbsb[k], in1=mA)
                    Bb = stat.tile([128, 1], BF16, tag=f"Bb{k}")
                    nc.vector.tensor_copy(out=Bb, in_=Bt)
                    Af.append(At)
                    Bbf.append(Bb)

                # fold normalization into weights:
                #   W' = A (row) ∘ W ;  b' = b + B @ W
                # B@W matmuls read the ORIGINAL W (Tile orders them before the
                # in-place row scale below via WAR deps).
                badj = {}
                for n in ("wq", "wk", "wv"):
                    pb = psm.tile([1, 512], F32, tag="pb", name=f"pb_{n}")
                    for k in range(CT):
                        nc.tensor.matmul(
                            pb, Bbf[k], wsb[n][k],
                            start=(k == 0), stop=(k == CT - 1),
                        )
                    bs_ = stat.tile([1, 512], F32, tag=f"badj_{n}")
                    nc.vector.tensor_copy(out=bs_, in_=pb)
                    badj[n] = bs_
                # row-scale the weights on the Scalar engine (idle here) into
                # SEPARATE tiles: an in-place fold would WAR-depend on the
                # b-adjust matmuls above (which need the last tile's stats),
                # serializing the whole prologue. Separate outputs let the
                # k-th fold fire as soon as A[k] is ready, so QKV matmuls
                # start ~25us earlier.
                wf = {}
                for n in ("wq", "wk", "wv"):
                    wf[n] = []
                    for k in range(CT):
                        t = const.tile([128, C], BF16, tag=f"wf_{n}_{k}",
                                       name=f"wf_{n}_{k}")
                        nc.scalar.activation(
                            out=t, in_=wsb[n][k],
                            func=mybir.ActivationFunctionType.Copy,
                            scale=Af[k],
                        )
                        wf[n].append(t)
                wf["wo"] = wsb["wo"]
                # transpose b' pieces to per-partition layout for q/k;
                # build broadcast bias for v.
                bqf, bkf = [], []
                for n, dst in (("wq", bqf), ("wk", bkf)):
                    for m in range(CT):
                        pt = psm.tile([128, 1], F32, tag="pt", name=f"pt_{n}{m}")
                        nc.tensor.matmul(
                            pt, badj[n][0:1, m * 128:(m + 1) * 128], onef,
                            start=True, stop=True,
                        )
                        bf = stat.tile([128, 1], F32, tag=f"bf_{n}{m}")
                        base = bsb["bq"][m] if n == "wq" else bsb["bk"][m]
                        nc.vector.tensor_add(out=bf, in0=pt, in1=base)
                        dst.append(bf)
                pvb = psm.tile([128, 512], F32, tag="pvb")
                nc.tensor.matmul(pvb, ones_bcol, badj["wv"], start=True, stop=True)
                nc.vector.tensor_add(out=bvb, in0=pvb, in1=bvb)

                # ---- projections (raw x in, folded weights) ----
                # q/k/v stored fp8-e4m3 in DoubleRow-interleaved layout
                # [128, ktile, free] (contraction index = ktile*128+partition)
                kT8 = big.tile([128, CT, N_TOK], FP8, tag="kT8")
                qT8 = big.tile([128, CT, NQ], FP8, tag="qT8")
                v_sb = big.tile([128, JT, C], FP8, tag="v")
                for m in range(CT):
                    for nt in range(N_TOK // 512):
                        pk = psp.tile([128, 512], F32, tag="p")
                        for k in range(CT):
                            nc.tensor.matmul(
                                pk,
                                wf["wk"][k][:, m * 128:(m + 1) * 128],
                                xT[k][:, nt * 512:(nt + 1) * 512],
                                start=(k == 0), stop=(k == CT - 1),
                            )
                        nc.vector.tensor_scalar_add(
                            out=kT8[:, m, nt * 512:(nt + 1) * 512],
                            in0=pk, scalar1=bkf[m],
                        )
                    for nt in range(NQ // 512):
                        pq = psp.tile([128, 512], F32, tag="p")
                        for k in range(CT):
                            nc.tensor.matmul(
                                pq,
                                wf["wq"][k][:, m * 128:(m + 1) * 128],
                                xT[k][:, nt * 512:(nt + 1) * 512],
                                start=(k == 0), stop=(k == CT - 1),
                            )
                        nc.vector.tensor_scalar_add(
                            out=qT8[:, m, nt * 512:(nt + 1) * 512],
                            in0=pq, scalar1=bqf[m],
                        )
                for jt in range(JT):
                    pv = psp.tile([128, 512], F32, tag="p")
                    for k in range(CT):
                        nc.tensor.matmul(
                            pv,
                            xT[k][:, jt * 128:(jt + 1) * 128],
                            wf["wv"][k],
                            start=(k == 0), stop=(k == CT - 1),
                        )
                    nc.vector.tensor_add(out=v_sb[:, jt, :], in0=pv, in1=bvb)

            # ---- attention ----
            with (
                tc.tile_pool(name="ps_att", bufs=4, space="PSUM") as psa,
                tc.tile_pool(name="ps_s", bufs=3, space="PSUM") as pss,
                tc.tile_pool(name="ps_o", bufs=1, space="PSUM") as pso,
            ):
                psd = pso  # den_ps is short-lived (j-loop end -> den_sb copy),
                # disjoint in time from the o/dT epilogue slots - share the bank
                NP_ = JT // 2  # j-tile pairs per i-block
                for ib in range(IB):
                    att_ps = [psa.tile([128, 512], F32, tag="att", name=f"att_ps{cs}")
                              for cs in range(CT)]
                    dacc = owork.tile([128, 512], F32, tag="dacc")
                    nc.vector.memset(dacc, 0.0)
                    qs = qT8[:, :, ib * 512:(ib + 1) * 512]
                    for g in range(NP_):
                        e_p = ework.tile([128, 2, 512], FP8, tag="e")
                        for o in range(2):
                            jt = 2 * g + o
                            s_ps = pss.tile([128, 512], F32, tag="s",
                                            name=f"s_ps{o}")
                            for kk in range(2):
                                nc.tensor.matmul(
                                    s_ps,
                                    kT8[:, 2 * kk:2 * kk + 2,
                                        jt * 128:(jt + 1) * 128],
                                    qs[:, 2 * kk:2 * kk + 2, :],
                                    start=(kk == 0), stop=(kk == 1),
                                    perf_mode=mybir.MatmulPerfMode.DoubleRow,
                                )
                            nc.scalar.activation(
                                out=e_p[:, o, :], in_=s_ps,
                                func=mybir.ActivationFunctionType.Exp,
                                scale=SCALE,
                            )
                            nc.vector.tensor_add(out=dacc, in0=dacc,
                                                 in1=e_p[:, o, :])
                        for cs in range(CT):
                            nc.tensor.matmul(
                                att_ps[cs],
                                v_sb[:, 2 * g:2 * g + 2,
                                     cs * 128:(cs + 1) * 128],
                                e_p,
                                start=(g == 0), stop=(g == NP_ - 1),
                                perf_mode=mybir.MatmulPerfMode.DoubleRow,
                            )
                    # denominator: column sums of dacc (over j partitions)
                    den_ps = psd.tile([1, 512], F32, tag="o",
                                      padded_shape=[128, 512], name="den_ps")
                    nc.tensor.matmul(den_ps, onesf_col, dacc, start=True, stop=True)
                    attT = []
                    for cs in range(CT):
                        t = attw.tile([128, 512], BF16, tag="attT", name=f"attT{cs}")
                        nc.vector.tensor_copy(out=t, in_=att_ps[cs])
                        attT.append(t)
                    den_sb = owork.tile([1, 512], F32, tag="den_sb")
                    nc.vector.tensor_copy(out=den_sb, in_=den_ps)
                    for it in range(4):
                        row0 = (ib * 4 + it) * 128
                        dT = pso.tile([128, 1], F32, tag="o",
                                      padded_shape=[128, 512], name=f"dT{it}")
                        nc.tensor.matmul(
                            dT, den_sb[0:1, it * 128:(it + 1) * 128], onef,
                            start=True, stop=True,
                        )
                        rden = rdenw.tile([128, 1], F32, tag="rden")
                        nc.vector.reciprocal(out=rden, in_=dT)
                        o_ps = pso.tile([128, 512], F32, tag="o", name=f"o_ps{it}")
                        for cs in range(CT):
                            nc.tensor.matmul(
                                o_ps,
                                attT[cs][:, it * 128:(it + 1) * 128],
                                wsb["wo"][cs],
                                start=(cs == 0), stop=(cs == CT - 1),
                            )
                        xr_t = xrw.tile([128, C], F32, tag="xr")
                        nc.sync.dma_start(
                            out=xr_t, in_=xr_e.ap()[row0:row0 + 128, :]
                        )
                        o_t = owork.tile([128, C], F32, tag="o")
                        nc.vector.scalar_tensor_tensor(
                            out=o_t, in0=o_ps, scalar=rden, in1=xr_t,
                            op0=mybir.AluOpType.mult, op1=mybir.AluOpType.add,
                        )
                        nc.sync.dma_start(
                            out=out_e.ap()[row0:row0 + 128, :], in_=o_t
                        )

    nc.compile()
    return nc


def _get_nc():
    if "nc" not in _CACHE:
        _CACHE["nc"] = _build()
    return _CACHE["nc"]


def kernel(**inputs) -> np.ndarray:
    x = np.asarray(inputs["x"], dtype=np.float32)          # [B,H,W,C]
    gn_scale = np.asarray(inputs["gn_scale"], np.float32)
    gn_bias = np.asarray(inputs["gn_bias"], np.float32)
    ws = {n: np.ascontiguousarray(
        np.asarray(inputs[n], np.float32).astype(ml_dtypes.bfloat16))
        for n in ("wq", "wk", "wv", "wo")}
    bs = {n: np.asarray(inputs[n], np.float32) for n in ("bq", "bk", "bv", "bo")}

    gmat = np.zeros((128, 128), np.float32)
    for g in range(128 // GS):
        gmat[g * GS:(g + 1) * GS, g * GS:(g + 1) * GS] = 1.0 / GS

    xf = x.reshape(B, N_TOK, C)
    in_maps = []
    for core in range(8):
        b, h = divmod(core, 2)
        own = xf[b, h * NQ:(h + 1) * NQ]          # [NQ, C] fp32
        other = xf[b, (1 - h) * NQ:(2 - h) * NQ]
        perm = np.concatenate([own, other], axis=0)        # own half first
        xt = np.ascontiguousarray(perm.T.astype(ml_dtypes.bfloat16))  # [C, N]
        xr = np.ascontiguousarray(own + bs["bo"][None, :])  # residual (+bo)
        in_maps.append({
            "xt": xt,
            "xr": xr,
            "wq": ws["wq"], "wk": ws["wk"], "wv": ws["wv"], "wo": ws["wo"],
            "bq": bs["bq"], "bk": bs["bk"], "bv": bs["bv"],
            "gsc": gn_scale, "gbi": gn_bias,
            "gmat": gmat,
        })

    nc = _get_nc()
    res = run_bass_kernel_spmd(nc, in_maps, core_ids=list(range(8)))

    out = np.empty((B, N_TOK, C), np.float32)
    for core in range(8):
        b, h = divmod(core, 2)
        out[b, h * NQ:(h + 1) * NQ] = res.results[core]["out"]
    return out.reshape(B, H, W, C)


# revision 27
# speedup vs baseline: 1.8575x; 1.1264x over previous
"""AttnBlock (GroupNorm + single-head self-attention + residual) on 8 TRN2
NeuronCores.

Reference computation (per image b of 4, tokens N=64*64=4096, C=512):
    hn  = GroupNorm(x)  (32 groups, eps 1e-6, affine)
    q,k,v = hn @ wq + bq, ...
    attn = softmax(q @ k.T / sqrt(C)); out = attn @ v
    y   = x + out @ wo + bo

Sharding: one NeuronCore per (image, half): core 2b+h computes attention
rows [h*2048, (h+1)*2048) of image b. Each core redundantly computes
GroupNorm stats and full-image K/V (cheap vs. cross-core collectives) and
its own 2048 query rows. No inter-core communication.

Layout: everything feature-major ([C, tokens]) so every matmul contraction
sits on the partition axis; the output projection naturally returns to
row-major. The host pre-transposes/casts x per core (shard prep) and
passes the residual rows in f32.

Device pipeline:
  1. GroupNorm statistics on the raw fp8 x, split across engines: two
     channel-tiles on DVE (bn_stats/bn_aggr), two on ACT (Copy/Square with
     accum_out; 1/N scales folded into the activation scale).
  2. The normalization hn = x*A + B is FOLDED INTO THE QKV WEIGHTS:
     W' = 16*A o W (row scale; the x16 keeps fp8 weights out of the
     subnormal range), b' = 16*(b + B@W). Projections consume raw x.
  3. q/k/v in fp8-e4m3, DoubleRow-interleaved [128, ktile, free]
     (contraction = ktile*128 + partition). All projection and attention
     matmuls run fp8 DoubleRow (2 contraction tiles per instruction).
  4. Attention: scoresT = k'^T q' (256x true scores), exp on ACT with the
     1/256 folded into the softmax scale, softmax denominator accumulated
     on DVE, attn@v in DoubleRow, output projection in bf16, the
     1/(16*den) applied per query row in the epilogue (softmax
     normalization commutes with the linear attn@v / output proj).
Scores are provably in [-2,2] for unit-normalized inputs so exp without
max subtraction is safe. PSUM accumulation is fp32 throughout.
"""

import sys

if "/opt/trn_rl_repo" not in sys.path:
    sys.path.insert(0, "/opt/trn_rl_repo")

import numpy as np
import ml_dtypes

import concourse.bass as bass
import concourse.tile as tile
from concourse import bacc, mybir
from concourse.bass_utils import run_bass_kernel_spmd

F32 = mybir.dt.float32
BF16 = mybir.dt.bfloat16
FP8 = mybir.dt.float8e4

B, H, W, C = 4, 64, 64, 512
N_TOK = H * W            # tokens per image
NQ = N_TOK // 2          # query rows per core
G = 32                   # groups
GS = C // G              # channels per group (16)
EPS = 1e-6
SCALE = float(C) ** -0.5
CT = C // 128            # channel tiles (4)
JT = N_TOK // 128        # token tiles (32)
IB = NQ // 512           # query i-blocks (4)
WS = 16.0                # fp8 weight scale (q,k,v carry a x16 factor)
DR = mybir.MatmulPerfMode.DoubleRow

_CACHE = {}


def _build():
    nc = bacc.Bacc("TRN2", target_bir_lowering=False)

    # x: fp8, DoubleRow-interleaved [128, ktile, tok]
    xt_e = nc.dram_tensor("xt", [128, CT, N_TOK], FP8, kind="ExternalInput")
    xr_e = nc.dram_tensor("xr", [NQ, C], F32, kind="ExternalInput")
    w_e = {
        n: nc.dram_tensor(n, [C, C], BF16, kind="ExternalInput")
        for n in ("wq", "wk", "wv", "wo")
    }
    b_e = {   # bq/bk pre-scaled x16 by the host; bv true-scale
        n: nc.dram_tensor(n, [C], F32, kind="ExternalInput")
        for n in ("bq", "bk", "bv")
    }
    gs_e = nc.dram_tensor("gsc", [C], F32, kind="ExternalInput")
    gb_e = nc.dram_tensor("gbi", [C], F32, kind="ExternalInput")
    gm_e = nc.dram_tensor("gmat", [128, 128], F32, kind="ExternalInput")
    out_e = nc.dram_tensor("out", [NQ, C], F32, kind="ExternalOutput")

    def col(e):  # [C] dram -> [C,1] view for partition-major loads
        return e.ap().rearrange("(a b) -> a b", b=1)

    with tile.TileContext(nc) as tc:
        with (
            tc.tile_pool(name="const", bufs=1) as const,
            tc.tile_pool(name="big", bufs=1) as big,
            tc.tile_pool(name="stat", bufs=1) as stat,
            tc.tile_pool(name="ework", bufs=6) as ework,
            tc.tile_pool(name="attw", bufs=8) as attw,
            tc.tile_pool(name="owork", bufs=4) as owork,
            tc.tile_pool(name="xrw", bufs=3) as xrw,
            tc.tile_pool(name="rdenw", bufs=8) as rdenw,
        ):
            # ---- DMA issue order = need order (per-queue completion
            # counters are cumulative): gmat/biases gate the first PE op,
            # x gates stats, weights are needed ~20us in.
            gm_sb = const.tile([128, 128], F32, tag="gmat")
            nc.sync.dma_start(out=gm_sb, in_=gm_e.ap())

            xT = big.tile([128, CT, N_TOK], FP8, tag="xT")
            for k in range(CT):
                nc.sync.dma_start(out=xT[:, k, :], in_=xt_e.ap()[:, k, :])

            bsb = {}
            for n in ("bq", "bk"):
                bsb[n] = []
                for m in range(CT):
                    t = const.tile([128, 1], F32, tag=f"b_{n}_{m}")
                    nc.sync.dma_start(out=t, in_=col(b_e[n])[m * 128:(m + 1) * 128, :])
                    bsb[n].append(t)
            bv_row = const.tile([1, C], F32, tag="bv_row")
            nc.sync.dma_start(out=bv_row, in_=b_e["bv"].ap()[None, :])
            gssb, gbsb = [], []
            for m in range(CT):
                t = const.tile([128, 1], F32, tag=f"gs_{m}")
                nc.sync.dma_start(out=t, in_=col(gs_e)[m * 128:(m + 1) * 128, :])
                gssb.append(t)
                t = const.tile([128, 1], F32, tag=f"gb_{m}")
                nc.sync.dma_start(out=t, in_=col(gb_e)[m * 128:(m + 1) * 128, :])
                gbsb.append(t)
            # ---- weights (largest, least urgent) ----
            wsb = {}
            for n in ("wq", "wk", "wv", "wo"):
                wsb[n] = []
                for k in range(CT):
                    t = const.tile([128, C], BF16, tag=f"w_{n}_{k}", name=f"w_{n}_{k}")
                    nc.sync.dma_start(out=t, in_=w_e[n].ap()[k * 128:(k + 1) * 128, :])
                    wsb[n].append(t)
            ones16_bcol = const.tile([1, 128], F32, tag="ones16_bcol")
            nc.vector.memset(ones16_bcol, WS)
            one16 = const.tile([1, 1], F32, tag="one16")
            nc.vector.memset(one16, WS)
            onesf_col = const.tile([128, 1], F32, tag="onesf_col")
            nc.vector.memset(onesf_col, 1.0)
            epst = const.tile([128, 1], F32, tag="epst")
            nc.vector.memset(epst, EPS)

            # ---- GroupNorm stats + weight folding + projections ----
            # ps_misc and ps_proj open simultaneously: a stacked open/close
            # would make the projection pool's banks WAR-depend on the whole
            # stats phase.
            with (
                tc.tile_pool(name="ps_misc", bufs=1, space="PSUM") as psm,
                tc.tile_pool(name="ps_proj", bufs=4, space="PSUM") as psp,
            ):
                # per-channel (mean, E[x^2]): tiles 0,3 on DVE (first/last
                # DMA arrivals), tiles 1,2 on ACT.
                scr0 = stat.tile([128, N_TOK], BF16, tag="scr", name="scr0")
                scr = [scr0, scr0]
                sms = []
                for k in range(CT):
                    sm = stat.tile([128, 2], F32, tag=f"sm{k}", name=f"sm{k}")
                    if k in (1, 2):
                        nc.scalar.activation(
                            out=scr[k - 1], in_=xT[:, k, :],
                            func=mybir.ActivationFunctionType.Copy,
                            scale=1.0 / N_TOK, accum_out=sm[:, 0:1],
                        )
                        nc.scalar.activation(
                            out=scr[k - 1], in_=xT[:, k, :],
                            func=mybir.ActivationFunctionType.Square,
                            scale=1.0 / float(np.sqrt(N_TOK)),
                            accum_out=sm[:, 1:2],
                        )
                    else:
                        stats = stat.tile([128, 8, 6], F32, tag=f"st{k}",
                                          name=f"st{k}")
                        for ch in range(8):
                            nc.vector.bn_stats(
                                out=stats[:, ch, :],
                                in_=xT[:, k, ch * 512:(ch + 1) * 512],
                            )
                        mv = stat.tile([128, 2], F32, tag=f"mv{k}", name=f"mv{k}")
                        nc.vector.bn_aggr(out=mv, in_=stats)
                        nc.vector.tensor_copy(out=sm[:, 0:1], in_=mv[:, 0:1])
                        nc.vector.tensor_mul(out=sm[:, 1:2], in0=mv[:, 0:1],
                                             in1=mv[:, 0:1])
                        nc.vector.tensor_add(out=sm[:, 1:2], in0=sm[:, 1:2],
                                             in1=mv[:, 1:2])
                    sms.append(sm)

                Af16, Bbf = [], []
                for k in range(CT):
                    # group-average via block-diagonal (1/GS) matrix
                    gps = psm.tile([128, 2], F32, tag="gps", name=f"gps{k}")
                    nc.tensor.matmul(gps, gm_sb, sms[k], start=True, stop=True)
                    gsb = stat.tile([128, 2], F32, tag=f"gsb{k}")
                    nc.vector.tensor_copy(out=gsb, in_=gps)
                    # var_g = E[x^2]-mean^2; A = rstd*scale; B = bias-mean*A
                    msq = stat.tile([128, 1], F32, tag=f"msq{k}")
                    nc.vector.tensor_mul(out=msq, in0=gsb[:, 0:1], in1=gsb[:, 0:1])
                    varg = stat.tile([128, 1], F32, tag=f"vg{k}")
                    nc.vector.tensor_sub(out=varg, in0=gsb[:, 1:2], in1=msq)
                    sd = stat.tile([128, 1], F32, tag=f"sd{k}")
                    nc.scalar.activation(
                        out=sd, in_=varg,
                        func=mybir.ActivationFunctionType.Sqrt,
                        bias=epst, scale=1.0,
                    )
                    rstd = stat.tile([128, 1], F32, tag=f"rs{k}")
                    nc.vector.reciprocal(out=rstd, in_=sd)
                    At = stat.tile([128, 1], F32, tag=f"A{k}")
                    nc.vector.tensor_mul(out=At, in0=rstd, in1=gssb[k])
                    A16 = stat.tile([128, 1], F32, tag=f"A16_{k}")
                    nc.vector.tensor_scalar_mul(out=A16, in0=At, scalar1=WS)
                    mA = stat.tile([128, 1], F32, tag=f"mA{k}")
                    nc.vector.tensor_mul(out=mA, in0=gsb[:, 0:1], in1=At)
                    Bt = stat.tile([128, 1], F32, tag=f"B{k}")
                    nc.vector.tensor_sub(out=Bt, in0=gbsb[k], in1=mA)
                    Bb = stat.tile([128, 1], BF16, tag=f"Bb{k}")
                    nc.vector.tensor_copy(out=Bb, in_=Bt)
                    Af16.append(A16)
                    Bbf.append(Bb)

                # b-adjust: badj_n = B @ W_n (true scale, from original W)
                badj = {}
                for n in ("wq", "wk", "wv"):
                    pb = psm.tile([1, 512], F32, tag="pb", name=f"pb_{n}")
                    for k in range(CT):
                        nc.tensor.matmul(
                            pb, Bbf[k], wsb[n][k],
                            start=(k == 0), stop=(k == CT - 1),
                        )
                    bs_ = stat.tile([1, 512], F32, tag=f"badj_{n}")
                    nc.vector.tensor_copy(out=bs_, in_=pb)
                    badj[n] = bs_
                # fold: W' = 16*A o W, into fp8 DoubleRow layout, split
                # across ACT/DVE. Separate output tiles so fold k fires as
                # soon as A16[k] is ready (an in-place fold would WAR-wait
                # on the b-adjust matmuls above).
                wf = {}
                for n in ("wq", "wk", "wv"):
                    wt = big.tile([128, CT, C], FP8, tag=f"wf_{n}", name=f"wf_{n}")
                    for k in range(CT):
                        if k in (1, 2):
                            nc.scalar.activation(
                                out=wt[:, k, :], in_=wsb[n][k],
                                func=mybir.ActivationFunctionType.Copy,
                                scale=Af16[k],
                            )
                        else:
                            nc.vector.tensor_scalar_mul(
                                out=wt[:, k, :], in0=wsb[n][k], scalar1=Af16[k],
                            )
                    wf[n] = wt
                # per-partition biases for q/k: bf = 16*badj + 16*b
                # (b arrives pre-scaled x16 from the host; the one16 rhs
                # applies the x16 to badj during the transpose matmul)
                bqf, bkf = [], []
                for n, dst in (("wq", bqf), ("wk", bkf)):
                    for m in range(CT):
                        pt = psm.tile([128, 1], F32, tag="pt", name=f"pt_{n}{m}")
                        nc.tensor.matmul(
                            pt, badj[n][0:1, m * 128:(m + 1) * 128], one16,
                            start=True, stop=True,
                        )
                        bf = stat.tile([128, 1], F32, tag=f"bf_{n}{m}")
                        base = bsb["bq"][m] if n == "wq" else bsb["bk"][m]
                        nc.vector.tensor_add(out=bf, in0=pt, in1=base)
                        dst.append(bf)
                # v bias broadcast: bvb16 = 16*(bv + B@wv) over 128 rows
                # (the x16 comes from the ones16 lhsT of the broadcast)
                bveff = stat.tile([1, C], F32, tag="bveff")
                nc.vector.tensor_add(out=bveff, in0=badj["wv"], in1=bv_row)
                pvb = psm.tile([128, 512], F32, tag="pvb")
                nc.tensor.matmul(pvb, ones16_bcol, bveff, start=True, stop=True)
                bvb16 = const.tile([128, C], F32, tag="bvb16")
                nc.vector.tensor_copy(out=bvb16, in_=pvb)

                # ---- projections: fp8 DoubleRow, raw x in ----
                kT8 = big.tile([128, CT, N_TOK], FP8, tag="kT8")
                qT8 = big.tile([128, CT, NQ], FP8, tag="qT8")
                v_sb = big.tile([128, JT, C], FP8, tag="v")
                for m in range(CT):
                    for nt in range(N_TOK // 512):
                        pk = psp.tile([128, 512], F32, tag="p")
                        for kk in range(2):
                            nc.tensor.matmul(
                                pk,
                                wf["wk"][:, 2 * kk:2 * kk + 2,
                                         m * 128:(m + 1) * 128],
                                xT[:, 2 * kk:2 * kk + 2,
                                   nt * 512:(nt + 1) * 512],
                                start=(kk == 0), stop=(kk == 1),
                                perf_mode=DR,
                            )
                        # k/q PSUM evacuation on ACT (Identity fuses the
                        # per-partition bias); v evacuation stays on DVE.
                        nc.scalar.activation(
                            out=kT8[:, m, nt * 512:(nt + 1) * 512], in_=pk,
                            func=mybir.ActivationFunctionType.Identity,
                            bias=bkf[m], scale=1.0,
                        )
                    for nt in range(NQ // 512):
                        pq = psp.tile([128, 512], F32, tag="p")
                        for kk in range(2):
                            nc.tensor.matmul(
                                pq,
                                wf["wq"][:, 2 * kk:2 * kk + 2,
                                         m * 128:(m + 1) * 128],
                                xT[:, 2 * kk:2 * kk + 2,
                                   nt * 512:(nt + 1) * 512],
                                start=(kk == 0), stop=(kk == 1),
                                perf_mode=DR,
                            )
                        nc.scalar.activation(
                            out=qT8[:, m, nt * 512:(nt + 1) * 512], in_=pq,
                            func=mybir.ActivationFunctionType.Identity,
                            bias=bqf[m], scale=1.0,
                        )
                for jt in range(JT):
                    pv = psp.tile([128, 512], F32, tag="p")
                    for kk in range(2):
                        nc.tensor.matmul(
                            pv,
                            xT[:, 2 * kk:2 * kk + 2, jt * 128:(jt + 1) * 128],
                            wf["wv"][:, 2 * kk:2 * kk + 2, :],
                            start=(kk == 0), stop=(kk == 1),
                            perf_mode=DR,
                        )
                    nc.vector.tensor_add(out=v_sb[:, jt, :], in0=pv, in1=bvb16)

            # ---- attention ----
            with (
                tc.tile_pool(name="ps_att", bufs=4, space="PSUM") as psa,
                tc.tile_pool(name="ps_s", bufs=3, space="PSUM") as pss,
                tc.tile_pool(name="ps_o", bufs=1, space="PSUM") as pso,
            ):
                psd = pso  # den_ps is short-lived; share the epilogue bank
                NP_ = JT // 2  # j-tile pairs per i-block
                for ib in range(IB):
                    att_ps = [psa.tile([128, 512], F32, tag="att", name=f"att_ps{cs}")
                              for cs in range(CT)]
                    dacc = owork.tile([128, 512], F32, tag="dacc")
                    nc.vector.memset(dacc, 0.0)
                    qs = qT8[:, :, ib * 512:(ib + 1) * 512]
                    for g in range(NP_):
                        e_p = ework.tile([128, 2, 512], FP8, tag="e")
                        for o in range(2):
                            jt = 2 * g + o
                            s_ps = pss.tile([128, 512], F32, tag="s",
                                            name=f"s_ps{o}")
                            for kk in range(2):
                                nc.tensor.matmul(
                                    s_ps,
                                    kT8[:, 2 * kk:2 * kk + 2,
                                        jt * 128:(jt + 1) * 128],
                                    qs[:, 2 * kk:2 * kk + 2, :],
                                    start=(kk == 0), stop=(kk == 1),
                                    perf_mode=DR,
                                )
                            # scores carry 16*16 = 256x; fold into exp scale
                            nc.scalar.activation(
                                out=e_p[:, o, :], in_=s_ps,
                                func=mybir.ActivationFunctionType.Exp,
                                scale=SCALE / (WS * WS),
                            )
                            nc.vector.tensor_add(out=dacc, in0=dacc,
                                                 in1=e_p[:, o, :])
                        for cs in range(CT):
                            nc.tensor.matmul(
                                att_ps[cs],
                                v_sb[:, 2 * g:2 * g + 2,
                                     cs * 128:(cs + 1) * 128],
                                e_p,
                                start=(g == 0), stop=(g == NP_ - 1),
                                perf_mode=DR,
                            )
                    # denominator: column sums of dacc (over j partitions)
                    den_ps = psd.tile([1, 512], F32, tag="o",
                                      padded_shape=[128, 512], name="den_ps")
                    nc.tensor.matmul(den_ps, onesf_col, dacc, start=True, stop=True)
                    attT = []
                    for cs in range(CT):
                        t = attw.tile([128, 512], BF16, tag="attT", name=f"attT{cs}")
                        nc.vector.tensor_copy(out=t, in_=att_ps[cs])
                        attT.append(t)
                    den_sb = owork.tile([1, 512], F32, tag="den_sb")
                    nc.vector.tensor_copy(out=den_sb, in_=den_ps)
                    # transpose den to per-partition layout, batched:
                    # dTa[:, it] = 16*den[it*128:(it+1)*128]; one reciprocal
                    # gives rden = 1/(16 den) (the v path carries x16).
                    dTa = pso.tile([128, 4], F32, tag="o",
                                   padded_shape=[128, 512], name="dTa")
                    for it in range(4):
                        nc.tensor.matmul(
                            dTa[:, it:it + 1],
                            den_sb[0:1, it * 128:(it + 1) * 128], one16,
                            start=True, stop=True,
                        )
                    rden_all = rdenw.tile([128, 4], F32, tag="rden")
                    nc.vector.reciprocal(out=rden_all, in_=dTa)
                    for it in range(4):
                        row0 = (ib * 4 + it) * 128
                        o_ps = pso.tile([128, 512], F32, tag="o", name=f"o_ps{it}")
                        for cs in range(CT):
                            nc.tensor.matmul(
                                o_ps,
                                attT[cs][:, it * 128:(it + 1) * 128],
                                wsb["wo"][cs],
                                start=(cs == 0), stop=(cs == CT - 1),
                            )
                        xr_t = xrw.tile([128, C], F32, tag="xr")
                        nc.sync.dma_start(
                            out=xr_t, in_=xr_e.ap()[row0:row0 + 128, :]
                        )
                        o_t = owork.tile([128, C], F32, tag="o")
                        nc.vector.scalar_tensor_tensor(
                            out=o_t, in0=o_ps, scalar=rden_all[:, it:it + 1],
                            in1=xr_t,
                            op0=mybir.AluOpType.mult, op1=mybir.AluOpType.add,
                        )
                        nc.sync.dma_start(
                            out=out_e.ap()[row0:row0 + 128, :], in_=o_t
                        )

    nc.compile()
    return nc


def _get_nc():
    if "nc" not in _CACHE:
        _CACHE["nc"] = _build()
    return _CACHE["nc"]


def kernel(**inputs) -> np.ndarray:
    x = np.asarray(inputs["x"], dtype=np.float32)          # [B,H,W,C]
    gn_scale = np.asarray(inputs["gn_scale"], np.float32)
    gn_bias = np.asarray(inputs["gn_bias"], np.float32)
    ws = {n: np.ascontiguousarray(
        np.asarray(inputs[n], np.float32).astype(ml_dtypes.bfloat16))
        for n in ("wq", "wk", "wv", "wo")}
    bs = {n: np.asarray(inputs[n], np.float32) for n in ("bq", "bk", "bv", "bo")}

    gmat = np.zeros((128, 128), np.float32)
    for g in range(128 // GS):
        gmat[g * GS:(g + 1) * GS, g * GS:(g + 1) * GS] = 1.0 / GS

    xf = x.reshape(B, N_TOK, C)
    in_maps = []
    for core in range(8):
        b, h = divmod(core, 2)
        own = xf[b, h * NQ:(h + 1) * NQ]          # [NQ, C] fp32
        other = xf[b, (1 - h) * NQ:(2 - h) * NQ]
        perm = np.concatenate([own, other], axis=0)        # own half first
        # fp8, feature-major, DoubleRow-interleaved [128, CT, N_TOK]
        xt = np.ascontiguousarray(
            perm.T.reshape(CT, 128, N_TOK).transpose(1, 0, 2)
            .astype(ml_dtypes.float8_e4m3))
        xr = np.ascontiguousarray(own + bs["bo"][None, :])  # residual (+bo)
        in_maps.append({
            "xt": xt,
            "xr": xr,
            "wq": ws["wq"], "wk": ws["wk"], "wv": ws["wv"], "wo": ws["wo"],
            "bq": bs["bq"] * WS, "bk": bs["bk"] * WS, "bv": bs["bv"],
            "gsc": gn_scale, "gbi": gn_bias,
            "gmat": gmat,
        })

    nc = _get_nc()
    res = run_bass_kernel_spmd(nc, in_maps, core_ids=list(range(8)))

    out = np.empty((B, N_TOK, C), np.float32)
    for core in range(8):
        b, h = divmod(core, 2)
        out[b, h * NQ:(h + 1) * NQ] = res.results[core]["out"]
    return out.reshape(B, H, W, C)
